# revision 43
# baseline (speedup 1.0000x reference)
"""Trainium2 Bass kernel for MultiHeadLatentAttention (MLA), 8-core SPMD.

Sharding: data-parallel over batch (4) x tensor-parallel over heads (2).
Core c handles batch c//2 and heads (c%2)*8 .. +8. Each core computes its
partial output projection; the host sums the two TP partials per batch and
adds the (v-bias-folded) output bias.

Device layout is feature-on-partition / token-on-free throughout, so every
projection is a plain matmul chain with no transposes. Attention uses
transposed scores (keys on partitions) so probs feed the AV matmul directly.

v3 notes (vs the 350us v2):
- BOTH down-projections are token-split across the TP pair: each core
  computes all output chunks for its own 512 tokens only. kv-down PE work
  halves; per-core X is 2MB instead of 4MB so the first matmul fires ~4us in.
- three pipelined AllGathers: kv latent + kRot early (hidden under the qd
  chains), then the raw q latent in two 6-chunk halves (hidden under
  kNope/v, feeding the qu chains just in time).
- q sum-of-squares pre-accumulated on the DVE (2 ones-matmuls instead of 24)
- q rot-half via gpsimd DMA block swaps instead of PE permutation matmuls
- v2 carry-overs: rsqrt via Sqrt+fast-reciprocal, head-pair score pipelining,
  v-bias folded into bo on the host, 128x128 universal triangle mask.
"""

import sys
from contextlib import ExitStack

import numpy as np
import ml_dtypes

for _p in ("/opt/trn_rl_repo", "/root/.axon_site/_ro/trn_rl_repo"):
    if _p not in sys.path:
        sys.path.append(_p)

import concourse.bass as bass  # noqa: E402
import concourse.mybir as mybir  # noqa: E402
from concourse import bacc  # noqa: E402
from concourse.bass_utils import run_bass_kernel_spmd  # noqa: E402
from concourse.tile import TileContext  # noqa: E402

# Problem shapes (hardcoded per contract)
B, S, D = 4, 1024, 2048
H = 16
QL, KVL = 1536, 512
NOPE, ROPE, VH = 128, 64, 128
QKH = NOPE + ROPE  # 192
EPS = 1e-6

P = 128
T = S          # tokens per core (one batch)
TH = T // 2    # own-token half per core
DC = D // P    # 16 X chunks
QC = QL // P   # 12 q-latent chunks
KC = KVL // P  # 4 kv-latent chunks
HH = H // 2    # 8 heads per core
NKV = KVL + ROPE  # 576
NEG = -1.0e4   # mask bias (exp underflows to exactly 0)

f32 = mybir.dt.float32
bf16 = mybir.dt.bfloat16
AF = mybir.ActivationFunctionType


def build_nc(start: int):
    nc = bacc.Bacc(None, target_bir_lowering=False, debug=False)

    # all weights arrive host-swizzled to partition-major tile layout
    # [P, tile, c, m] so every DMA descriptor is 1-4KB contiguous.
    # xt token axis is host-reordered to [own half | peer half] so the
    # token-split kv path is SPMD-uniform; qd is out-dim split (6 chunks).
    dp = nc.declare_dram_parameter
    xt = dp("xt", [P, DC * T], bf16, isOutput=False)      # X[b].T reordered
    wqd = dp("wqd", [P, 6 * DC * P], bf16, isOutput=False)
    wkvd = dp("wkvd", [P, 4 * DC * P], bf16, isOutput=False)
    wkv5 = dp("wkv5", [P, DC * ROPE], bf16, isOutput=False)
    wqu = dp("wqu", [P, QC * QC * P], bf16, isOutput=False)
    wkn = dp("wkn", [P, HH * KC * P], bf16, isOutput=False)
    wv = dp("wv", [P, 4 * KC * 256], bf16, isOutput=False)
    wo = dp("wo", [P, DC * HH * P], bf16, isOutput=False)
    bqd_i = dp("bqd", [P, 6], f32, isOutput=False)        # qd bias (local)
    bkvd_i = dp("bkvd", [P, 5], f32, isOutput=False)      # kv down bias
    bqu_i = dp("bqu", [P, QC], f32, isOutput=False)       # perm + scale
    bkvuk = dp("bkvuk", [P, HH], f32, isOutput=False)     # kNope part
    cos2 = dp("cos2", [P, T], bf16, isOutput=False)       # q rope, dup rows
    sina = dp("sina", [P, T], bf16, isOutput=False)       # sign-folded sin
    cosk = dp("cosk", [ROPE, TH], bf16, isOutput=False)   # k rope own tokens
    sink = dp("sink", [ROPE, TH], bf16, isOutput=False)   # sign-folded
    tri_i = dp("tri", [P, P], bf16, isOutput=False)       # diag-band mask
    outt = dp("outt", [D, T], bf16, isOutput=True)

    # collectives: kv latent+kRot early, q latent in two 3-chunk halves
    cc1_in = nc.dram_tensor("cc1_in", [5 * P, TH], bf16)
    cc1_out = nc.dram_tensor("cc1_out", [10 * P, TH], bf16)
    cc2a_in = nc.dram_tensor("cc2a_in", [3 * P, T], bf16)
    cc2a_out = nc.dram_tensor("cc2a_out", [6 * P, T], bf16)
    cc2b_in = nc.dram_tensor("cc2b_in", [3 * P, T], bf16)
    cc2b_out = nc.dram_tensor("cc2b_out", [6 * P, T], bf16)
    RG = [[0, 1], [2, 3], [4, 5], [6, 7]]

    xt_r = xt.rearrange("p (c t) -> p c t", c=DC)
    wqd_r = wqd.rearrange("p (n c m) -> p n c m", n=6, c=DC)
    wkvd_r = wkvd.rearrange("p (n c m) -> p n c m", n=4, c=DC)
    wkv5_r = wkv5.rearrange("p (c m) -> p c m", c=DC)
    wqu_r = wqu.rearrange("p (n c m) -> p n c m", n=QC, c=QC)
    wkn_r = wkn.rearrange("p (n c m) -> p n c m", n=HH, c=KC)
    wv_r = wv.rearrange("p (n c m) -> p n c m", n=4, c=KC)
    wo_r = wo.rearrange("p (n c m) -> p n c m", n=DC, c=HH)
    outt_r = outt.rearrange("(c p) t -> p c t", p=P)
    cc1_in_r = cc1_in.rearrange("(c p) t -> p c t", p=P)
    cc1_out_r = cc1_out.rearrange("(r c p) t -> p r c t", p=P, r=2)
    cc2a_in_r = cc2a_in.rearrange("(c p) t -> p c t", p=P)
    cc2a_out_r = cc2a_out.rearrange("(r c p) t -> p r c t", p=P, r=2)
    cc2b_in_r = cc2b_in.rearrange("(c p) t -> p c t", p=P)
    cc2b_out_r = cc2b_out.rearrange("(r c p) t -> p r c t", p=P, r=2)

    with TileContext(nc) as tc, ExitStack() as stk:
        const = stk.enter_context(tc.tile_pool(name="const", bufs=1))
        persist = stk.enter_context(tc.tile_pool(name="persist", bufs=1))

        # ---- constants in SBUF ----
        c_bqd = const.tile([P, 6], f32)
        c_bkvd = const.tile([P, 5], f32)
        c_bqu = const.tile([P, QC], f32)
        c_bkvuk = const.tile([P, HH], f32)
        c_tri = const.tile([P, P], bf16)
        c_cos = const.tile([P, T], bf16)
        c_sin = const.tile([P, T], bf16)
        c_cosk = const.tile([ROPE, TH], bf16)
        c_sink = const.tile([ROPE, TH], bf16)
        ones_bf = const.tile([P, P], bf16)
        nc.vector.memset(ones_bf[:], 1.0)
        eps_c = const.tile([P, 1], f32)
        nc.vector.memset(eps_c[:], EPS)

        # ---- persistent activations ----
        t_q = persist.tile([P, QC, T], bf16)      # q heads (nope 0-7, rope+)
        t_kn = persist.tile([P, HH, T], bf16)     # kNope[feat, head, tok]
        t_v = persist.tile([P, T // P, HH * P], bf16)  # v[tok, tchunk, hv]
        t_kr = persist.tile([P, T], bf16)         # kRot full, rows dup
        t_ao = persist.tile([P, HH, T], bf16)     # attn out [vh, head, tok]
        rq = persist.tile([P, T], f32)            # q rms scale (per token)

        # ====== phases 1+2: projections ======
        with tc.tile_pool(name="ph1", bufs=1) as ph1, \
             tc.tile_pool(name="wstream", bufs=2) as wst, \
             tc.tile_pool(name="wqu_p", bufs=2) as wqp, \
             tc.tile_pool(name="wkvu_p", bufs=2) as wkp, \
             tc.tile_pool(name="tmp", bufs=2) as tmp, \
             tc.tile_pool(name="psA", bufs=6, space="PSUM") as psA, \
             tc.tile_pool(name="psR", bufs=1, space="PSUM") as psR:

            # local (own-token / own-chunk) results stage into the gathered
            # tiles and are later overwritten by the identical gathered data
            t_x = ph1.tile([P, DC, T], bf16, name="t_x")
            t_kv = ph1.tile([P, KC, T], bf16, name="t_kv")     # gathered kv
            t_qd = ph1.tile([P, QC, T], bf16, name="t_qd")     # gathered q
            t_kp = ph1.tile([ROPE, TH], bf16, name="t_kp")     # rope chunk
            rkv = ph1.tile([P, TH], f32, name="rkv")
            acc = ph1.tile([P, 2, TH], bf16, name="acc")       # q ss preacc

            # X chunks + first kv weights lead. gpsimd carries ONLY the
            # collective traffic (+tiny consts) so cc1 fires asap.
            w_kvd = []
            for m in range(4):
                w_kvd.append(wst.tile([P, DC, P], bf16, tag="wkvd", bufs=4,
                                      name="w_kvd"))
            wt5 = wst.tile([P, DC, ROPE], bf16, tag="wkv5", bufs=1,
                           name="wt5")

            def wload(eng, wt, view, pieces=4):
                nchunk = wt.shape[1]
                step = (nchunk + pieces - 1) // pieces
                for i in range(0, nchunk, step):
                    j = min(i + step, nchunk)
                    eng.dma_start(wt[:, i:j, :], view[:, i:j, :])

            # preload the scalar activation table off the rms critical chain
            nc.scalar.activation(rkv[:, 0:1], eps_c[:], AF.Sqrt,
                                 bias=eps_c[:], scale=1.0)

            # issue order per queue == arrival order. Strict priority:
            # kv-down weights + own-half X first (kvd gates collective 1),
            # peer-half X next, qd weights stream after, cos/sin late.
            A, Bh = slice(0, TH), slice(TH, T)

            def xload(eng, c0, c1, h):
                eng.dma_start(t_x[:, c0:c1, h], xt_r[:, c0:c1, h])

            nc.gpsimd.dma_start(c_bkvd[:], bkvd_i[:])
            nc.gpsimd.dma_start(c_cosk[:], cosk[:])
            nc.gpsimd.dma_start(c_sink[:], sink[:])
            # kvd-critical set only (own-half X + kv weights), 3 queues
            wload(nc.scalar, w_kvd[0], wkvd_r[:, 0])
            xload(nc.sync, 0, 2, A)
            xload(nc.gpsimd, 2, 4, A)
            xload(nc.sync, 4, 6, A)
            wload(nc.gpsimd, w_kvd[1], wkvd_r[:, 1])
            xload(nc.scalar, 6, 8, A)
            xload(nc.sync, 8, 10, A)
            wload(nc.scalar, w_kvd[2], wkvd_r[:, 2])
            xload(nc.gpsimd, 10, 12, A)
            wload(nc.sync, w_kvd[3], wkvd_r[:, 3])
            xload(nc.scalar, 12, 14, A)
            xload(nc.gpsimd, 14, 16, A)
            nc.gpsimd.dma_start(wt5[:, :, :ROPE], wkv5_r[:])
            nc.gpsimd.dma_start(c_bqd[:], bqd_i[:])
            # peer-half X (qd tt=1) after everything kvd needs
            xload(nc.scalar, 0, 4, Bh)
            xload(nc.sync, 4, 8, Bh)
            xload(nc.scalar, 8, 12, Bh)
            xload(nc.sync, 12, 16, Bh)
            nc.gpsimd.dma_start(c_bqu[:], bqu_i[:])
            nc.gpsimd.dma_start(c_bkvuk[:], bkvuk[:])
            nc.gpsimd.dma_start(c_tri[:], tri_i[:])

            def down_chain(wt, m_rows, bias_t, bcol, out_ap, h):
                # out[m_rows, h] = wt.T @ X[:, h] + bias
                ps = psA.tile([P, TH], f32, tag="ev", name="ps_ev")
                psm = ps[:m_rows, :]
                for c in range(DC):
                    nc.tensor.matmul(
                        psm, wt[:, c, :m_rows], t_x[:, c, h],
                        start=(c == 0), stop=(c == DC - 1),
                    )
                nc.vector.tensor_scalar_add(
                    out=out_ap[:m_rows, :], in0=psm,
                    scalar1=bias_t[:m_rows, bcol:bcol + 1])

            # ---- kv down for own tokens (5 chunks incl. rope) ----
            for m in range(4):
                down_chain(w_kvd[m], P, c_bkvd, m, t_kv[:, m, A], A)
            down_chain(wt5, ROPE, c_bkvd, 4, t_kp[:, :], A)

            # kv rms on own tokens (ss via 4 ones-matmuls, then
            # Sqrt + fast-reciprocal + in-place normalize)
            ps_ms = psR.tile([P, 2, TH], f32, tag="ms", name="ps_ms")
            for c in range(KC):
                sq = tmp.tile([P, TH], bf16, tag="sq")
                nc.vector.tensor_mul(sq[:], t_kv[:, c, A], t_kv[:, c, A])
                nc.tensor.matmul(ps_ms[:, 0, :], ones_bf[:], sq[:],
                                 start=(c == 0), stop=(c == KC - 1))
            with tc.high_priority():
                nc.scalar.activation(rkv[:], ps_ms[:, 0, :], AF.Sqrt,
                                     bias=eps_c[:], scale=1.0 / KVL)
                nc.vector.reciprocal_approx_fast(out=rkv[:], in_=rkv[:])
                for c in range(KC):
                    nc.vector.tensor_mul(t_kv[:, c, A], t_kv[:, c, A],
                                         rkv[:])
                # RoPE on own kPos (unsigned 32-row swap + sign-folded sin)
                swp = tmp.tile([P, TH], bf16, tag="swp", name="swp",
                               bufs=1)[:ROPE, :]
                nc.sync.dma_start(swp[0:32, :], t_kp[32:64, :])
                nc.sync.dma_start(swp[32:64, :], t_kp[0:32, :])
                nc.vector.tensor_mul(t_kr[0:ROPE, A], t_kp[:, :], c_cosk[:])
                nc.vector.tensor_mul(swp[:], swp[:], c_sink[:])
                nc.vector.tensor_add(t_kr[0:ROPE, A], t_kr[0:ROPE, A],
                                     swp[:])
                nc.sync.dma_start(t_kr[ROPE:P, A], t_kr[0:ROPE, A])

                # ---- collective 1: normalized kv latent + kRot ----
                for m in range(4):
                    nc.gpsimd.dma_start(cc1_in_r[:, m, :], t_kv[:, m, A])
                nc.gpsimd.dma_start(cc1_in_r[:, 4, :], t_kr[:, A])
                nc.gpsimd.collective_compute(
                    "AllGather", mybir.AluOpType.bypass,
                    replica_groups=RG,
                    ins=[cc1_in[:]], outs=[cc1_out[:]],
                )
                for r in range(2):
                    nc.gpsimd.dma_start(t_kv[:, 0:4, bass.ts(r, TH)],
                                        cc1_out_r[:, r, 0:4, :])
                    nc.gpsimd.dma_start(t_kr[:, bass.ts(r, TH)],
                                        cc1_out_r[:, r, 4, :])

            # ---- q down: local 6 out-chunks over the full (reordered)
            # token axis; exchange in two 3-chunk collectives ----
            def q_exchange(cin_r, cin, cout, cout_r, m0):
                for mm in range(3):
                    nc.gpsimd.dma_start(cin_r[:, mm, :],
                                        t_qd[:, m0 + mm, :])
                nc.gpsimd.collective_compute(
                    "AllGather", mybir.AluOpType.bypass,
                    replica_groups=RG,
                    ins=[cin[:]], outs=[cout[:]],
                )
                # rank0 rows = global chunks m0..m0+2 (natural tokens);
                # rank1 rows = chunks m0+6.. with token halves swapped
                # (their producer's xt was reordered own-first too)
                nc.gpsimd.dma_start(t_qd[:, m0:m0 + 3, :],
                                    cout_r[:, 0, :, :])
                nc.gpsimd.dma_start(t_qd[:, m0 + 6:m0 + 9, A],
                                    cout_r[:, 1, :, TH:])
                nc.gpsimd.dma_start(t_qd[:, m0 + 6:m0 + 9, Bh],
                                    cout_r[:, 1, :, 0:TH])

            for m in range(6):
                wt = wst.tile([P, DC, P], bf16, tag="wqd", bufs=2,
                              name="w_qd")
                eng = nc.scalar if m % 2 == 0 else nc.sync
                wload(eng, wt, wqd_r[:, m])
                for tt in range(2):
                    down_chain(wt, P, c_bqd, m, t_qd[:, m, bass.ts(tt, TH)],
                               bass.ts(tt, TH))
                if m == 2:
                    q_exchange(cc2a_in_r, cc2a_in, cc2a_out, cc2a_out_r, 0)
            nc.scalar.dma_start(c_cos[:], cos2[:])
            nc.scalar.dma_start(c_sin[:], sina[:])
            q_exchange(cc2b_in_r, cc2b_in, cc2b_out, cc2b_out_r, 3)

            # ---- kNope up-projection (bias add on scalar engine) ----
            kn_w = []
            for m in range(HH):
                wt = wkp.tile([P, KC, P], bf16, tag="wkn", bufs=4,
                              name="kn_w")
                nc.sync.dma_start(wt[:], wkn_r[:, m])
                kn_w.append(wt)
            for m in range(HH):
                wt = kn_w[m]
                for tt in range(2):
                    ps = psA.tile([P, TH], f32, tag="ev", name="ps_kn")
                    for c in range(KC):
                        nc.tensor.matmul(
                            ps, wt[:, c, :],
                            t_kv[:, c, bass.ts(tt, TH)],
                            start=(c == 0), stop=(c == KC - 1),
                        )
                    nc.scalar.activation(
                        t_kn[:, m, bass.ts(tt, TH)], ps, AF.Identity,
                        bias=c_bkvuk[:, m:m + 1])

            # ---- v up-projection (token-on-partition) ----
            for gg in range(4):
                wt = wkp.tile([P, KC, 256], bf16, tag="wv")
                (nc.sync if gg % 2 == 0 else nc.scalar).dma_start(
                    wt[:], wv_r[:, gg])
                for tcb in range(8):
                    ps = psA.tile([P, 256], f32, tag="ev", name="ps_v")
                    for c in range(KC):
                        nc.tensor.matmul(
                            ps,
                            t_kv[:, c, bass.ts(tcb, P)],
                            wt[:, c, :],
                            start=(c == 0), stop=(c == KC - 1),
                        )
                    nc.scalar.activation(
                        t_v[:, tcb, bass.ds(gg * 256, 256)], ps, AF.Copy)

            # ---- q rms from the gathered raw latent (DVE pre-accum) ----
            with tc.tile_wait_until(0.085):
                for tt in range(2):
                    hs = bass.ts(tt, TH)
                    for c in range(QC):
                        if c == 0:
                            nc.vector.tensor_mul(
                                acc[:, tt, :], t_qd[:, 0, hs], t_qd[:, 0, hs])
                        else:
                            sq = tmp.tile([P, TH], bf16, tag="sq")
                            nc.vector.tensor_mul(
                                sq[:], t_qd[:, c, hs], t_qd[:, c, hs])
                            nc.vector.tensor_add(
                                acc[:, tt, :], acc[:, tt, :], sq[:])
                ps_mq = psR.tile([P, 2, TH], f32, tag="ms", name="ps_mq")
                for tt in range(2):
                    hs = bass.ts(tt, TH)
                    nc.tensor.matmul(ps_mq[:, tt, :], ones_bf[:],
                                     acc[:, tt, :], start=True, stop=True)
                    nc.scalar.activation(rq[:, hs], ps_mq[:, tt, :],
                                         AF.Sqrt, bias=eps_c[:],
                                         scale=1.0 / QL)
                    nc.vector.reciprocal_approx_fast(out=rq[:, hs],
                                                     in_=rq[:, hs])

            # ---- q up-projection ----
            # post-processing of chunk m's psums is issued after chunk m+1's
            # matmul chains, so the PE never waits on the DVE stage tiles
            def qu_post(m, ps, tt):
                tsl = bass.ts(tt, TH)
                if m < 8:
                    qsb = tmp.tile([P, TH], bf16, tag="qsb", bufs=2)
                    nc.vector.tensor_mul(qsb[:], ps, rq[:, tsl])
                    nc.scalar.activation(
                        t_q[:, m, tsl], qsb, AF.Identity,
                        bias=c_bqu[:, m:m + 1],
                    )
                else:
                    sq = tmp.tile([P, TH], bf16, tag="ropestage",
                                  bufs=2)
                    nc.vector.tensor_mul(sq[:], ps, rq[:, tsl])
                    nc.vector.tensor_scalar_add(
                        out=sq[:], in0=sq, scalar1=c_bqu[:, m:m + 1],
                    )
                    # rotate-half via gpsimd DMA 32-row block swaps
                    swb = tmp.tile([P, TH], bf16, tag="ropeswap",
                                   bufs=2)
                    nc.gpsimd.dma_start(swb[0:32, :], sq[32:64, :])
                    nc.gpsimd.dma_start(swb[32:64, :], sq[0:32, :])
                    nc.gpsimd.dma_start(swb[64:96, :], sq[96:128, :])
                    nc.gpsimd.dma_start(swb[96:128, :], sq[64:96, :])
                    qc = tmp.tile([P, TH], bf16, tag="ropecos", bufs=2)
                    nc.vector.tensor_mul(qc[:], sq[:], c_cos[:, tsl])
                    nc.vector.tensor_mul(swb[:], swb[:], c_sin[:, tsl])
                    nc.vector.tensor_add(t_q[:, m, tsl], qc[:], swb[:])

            pend = None
            for m in (8, 0, 1, 9, 2, 3, 10, 4, 5, 11, 6, 7):
                wt = wqp.tile([P, QC, P], bf16, tag="wqu")
                eng = nc.scalar if m % 2 == 0 else nc.sync
                wload(eng, wt, wqu_r[:, m])
                cur = []
                # contract the cc2a-delivered chunks first so the chains
                # tolerate a late cc2b readback
                CORD = (0, 1, 2, 6, 7, 8, 3, 4, 5, 9, 10, 11)
                for tt in range(2):
                    tsl = bass.ts(tt, TH)
                    ps = psA.tile([P, TH], f32, tag="ev", name="ps_qu")
                    for i, c in enumerate(CORD):
                        nc.tensor.matmul(
                            ps, wt[:, c, :], t_qd[:, c, tsl],
                            start=(i == 0), stop=(i == QC - 1),
                        )
                    cur.append(ps)
                if pend is not None:
                    pm, pps = pend
                    for tt in range(2):
                        qu_post(pm, pps[tt], tt)
                pend = (m, cur)
            pm, pps = pend
            for tt in range(2):
                qu_post(pm, pps[tt], tt)

        # ====== phase 3: attention (transposed scores, max-free) ======
        def vis_kcs(qt):
            return [kc for kc in range(8)
                    if qt * TH + TH - 1 >= kc * P - start]

        with tc.tile_pool(name="att", bufs=2) as att, \
             tc.tile_pool(name="psS", bufs=2, space="PSUM") as psS, \
             tc.tile_pool(name="psD", bufs=1, space="PSUM") as psD, \
             tc.tile_pool(name="psU", bufs=2, space="PSUM") as psU:

            def scores_qt(hp, expts2, qt, kcs=None):
                # expts2 [P, head2, kc, q] for heads (2hp, 2hp+1)
                rc = 8 + hp
                for kc in (vis_kcs(qt) if kcs is None else kcs):
                    lo = max(qt * TH, kc * P - start)
                    w = qt * TH + TH - lo
                    rel = lo - qt * TH
                    sc2 = psS.tile([P, 2, TH], f32, tag="sc", name="sc2")
                    for h2 in range(2):
                        h = 2 * hp + h2
                        nc.tensor.matmul(
                            sc2[:, h2, rel:],
                            t_kn[:, h, bass.ts(kc, P)],
                            t_q[:, h, bass.ds(lo, w)],
                            start=True, stop=False,
                        )
                    for h2 in range(2):
                        r0 = h2 * ROPE
                        nc.tensor.matmul(
                            sc2[:, h2, rel:],
                            t_kr[r0:r0 + ROPE, bass.ts(kc, P)],
                            t_q[r0:r0 + ROPE, rc, bass.ds(lo, w)],
                            start=False, stop=True,
                        )
                    # partially-masked diagonal band
                    b_lo = max(lo, kc * P - start)
                    b_hi = min(qt * TH + TH, kc * P - start + P)
                    bw = b_hi - b_lo
                    if bw > 0:
                        j0 = b_lo - (kc * P - start)
                        br = b_lo - qt * TH
                        for h2 in range(2):
                            nc.vector.tensor_add(
                                sc2[:, h2, br:br + bw],
                                sc2[:, h2, br:br + bw],
                                c_tri[:, j0:j0 + bw])
                    nc.scalar.activation(
                        expts2[:, :, kc, bass.ds(lo, w)],
                        sc2[:, :, rel:], AF.Exp)

            def den_head(hp, expts2, h2):
                den2 = psD.tile([P, 2, TH], f32, name="den2")
                for qt in range(2):
                    kcs = vis_kcs(qt)
                    for i, kc in enumerate(kcs):
                        lo = max(qt * TH, kc * P - start)
                        rel = lo - qt * TH
                        nc.tensor.matmul(
                            den2[:, qt, rel:], ones_bf[:],
                            expts2[:, h2, kc, bass.ds(lo, TH - rel)],
                            start=(i == 0), stop=(i == len(kcs) - 1),
                        )
                rcp = att.tile([P, 2, TH], f32, tag="rcp", name="rcp")
                nc.vector.reciprocal_approx_fast(
                    out=rcp[:, :, :], in_=den2[:, :, :])
                return rcp

            def outU_head(hp, expts2, h2, rcp):
                h = 2 * hp + h2
                for qt in range(2):
                    kcs = vis_kcs(qt)
                    outU = psU.tile([P, TH], f32, tag="outU", name="outU")
                    for i, kc in enumerate(kcs):
                        lo = max(qt * TH, kc * P - start)
                        rel = lo - qt * TH
                        nc.tensor.matmul(
                            outU[:, rel:], t_v[:, kc, bass.ts(h, P)],
                            expts2[:, h2, kc, bass.ds(lo, TH - rel)],
                            start=(i == 0), stop=(i == len(kcs) - 1),
                        )
                    nc.vector.tensor_mul(
                        t_ao[:, h, bass.ts(qt, TH)], outU[:],
                        rcp[:, qt, :])

            # interleave hp-1's den/outU chains between hp's score bursts
            # so the PE has filler while the exp stream catches up
            prev = None
            for hp in range(4):
                cur = att.tile([P, 2, 8, T], bf16, tag="expt", name="expt2")
                scores_qt(hp, cur, 0)
                if prev is not None:
                    rcp0 = den_head(hp - 1, prev, 0)
                scores_qt(hp, cur, 1, kcs=[0, 1, 2, 3])
                if prev is not None:
                    outU_head(hp - 1, prev, 0, rcp0)
                scores_qt(hp, cur, 1, kcs=[4, 5, 6, 7])
                if prev is not None:
                    rcp1 = den_head(hp - 1, prev, 1)
                    outU_head(hp - 1, prev, 1, rcp1)
                prev = cur
            for h2 in range(2):
                rcpt = den_head(3, prev, h2)
                outU_head(3, prev, h2, rcpt)

            # ====== phase 4: output projection ======
            for m in range(DC):
                wt = att.tile([P, HH, P], bf16, tag="wo", name="wo_t",
                              bufs=4)
                eng = nc.gpsimd if m % 2 == 0 else nc.sync
                eng.dma_start(wt[:], wo_r[:, m])
                for tt in range(2):
                    ps = psU.tile([P, TH], f32, tag="outU", name="ps_o")
                    for c in range(HH):
                        nc.tensor.matmul(
                            ps, wt[:, c, :], t_ao[:, c, bass.ts(tt, TH)],
                            start=(c == 0), stop=(c == HH - 1),
                        )
                    ot = att.tile([P, TH], bf16, tag="ot", name="ot",
                                  bufs=3)
                    nc.vector.tensor_copy(ot[:], ps)
                    oeng = nc.scalar if (2 * m + tt) % 2 == 0 else nc.sync
                    oeng.dma_start(outt_r[:, m, bass.ts(tt, TH)], ot[:])

    nc.compile()
    return nc


_CACHE = {}


def _get_nc(start: int):
    if start not in _CACHE:
        _CACHE[start] = build_nc(start)
    return _CACHE[start]


def _prep_inputs(X, base_freq, Wqd, bqd, gq, Wqu, bqu, Wkv, bkv, gkv,
                 Wkvu, bkvu, Wo, bo, start):
    f = np.float32
    X = np.asarray(X, f)
    base_freq = np.asarray(base_freq, f)
    Wqd = np.asarray(Wqd, f); bqd = np.asarray(bqd, f)
    gq = np.asarray(gq, f); Wqu = np.asarray(Wqu, f); bqu = np.asarray(bqu, f)
    Wkv = np.asarray(Wkv, f); bkv = np.asarray(bkv, f)
    gkv = np.asarray(gkv, f); Wkvu = np.asarray(Wkvu, f)
    bkvu = np.asarray(bkvu, f)
    Wo = np.asarray(Wo, f); bo = np.asarray(bo, f)
    start = int(np.asarray(start).item())
    assert start >= 0

    scale = QKH ** (-0.5)
    bf = ml_dtypes.bfloat16

    # v-bias exact fold: probs sum to 1, so the v bias contributes
    # Wo @ bv to every token's output.
    bv = bkvu.reshape(H, NOPE + VH)[:, NOPE:].reshape(H * VH)
    bo_eff = bo + Wo @ bv

    def _sw(wt, nt, c, m):
        # [c*P, nt*m] -> partition-major tiles [P, nt*c*m]
        a = np.asarray(wt, f).reshape(c, P, nt, m)
        a = np.ascontiguousarray(a.transpose(1, 2, 0, 3)).astype(bf)
        return a.reshape(P, nt * c * m)

    # qd is out-dim split: each core gets its group's 6 chunks
    wqd_g = [_sw(Wqd.T[:, g * 768:(g + 1) * 768], 6, DC, P) for g in range(2)]
    bqd_g = [np.ascontiguousarray(bqd[g * 768:(g + 1) * 768].reshape(6, P).T)
             for g in range(2)]
    wkv_t = Wkv.T.astype(f)                                   # (D, NKV)
    wkvd = _sw(wkv_t[:, :512], 4, DC, P)
    wkv5 = _sw(wkv_t[:, 512:576], 1, DC, ROPE)
    bkvd_p = np.zeros((5 * P,), f); bkvd_p[:NKV] = bkv
    bkvd = np.ascontiguousarray(bkvd_p.reshape(5, P).T)

    ang = base_freq[:S]                                       # (S, ROPE)
    cos = np.ascontiguousarray(np.cos(ang).T.astype(f))       # (ROPE, S)
    sin = np.ascontiguousarray(np.sin(ang).T.astype(f))
    cos2 = np.ascontiguousarray(
        np.concatenate([cos, cos], 0)).astype(bf)             # (128, S)
    sgn = np.ones((ROPE, 1), f); sgn[:ROPE // 2] = -1.0
    sins = sin * sgn                                          # sign-folded
    sina = np.ascontiguousarray(np.concatenate([sins, sins], 0)).astype(bf)

    # universal diagonal-band mask: for the block at k = kc*P + p,
    # q = (kc*P - start) + j, visibility is p <= j.
    pp = np.arange(P)
    tri = np.where(pp[:, None] <= pp[None, :], 0.0, NEG).astype(bf)
    tri = np.ascontiguousarray(tri)

    # per head-group tensors
    perm_q = np.concatenate(
        [np.arange(h * QKH, h * QKH + NOPE) for h in range(HH)]
        + [np.arange(h * QKH + NOPE, (h + 1) * QKH) for h in range(HH)]
    )
    perm_kv = np.concatenate(
        [np.arange(h * (NOPE + VH), h * (NOPE + VH) + NOPE) for h in range(HH)]
        + [np.arange(h * (NOPE + VH) + NOPE, (h + 1) * (NOPE + VH))
           for h in range(HH)]
    )
    gmaps = []
    for g in range(2):
        rq_ = slice(g * HH * QKH, (g + 1) * HH * QKH)
        rkv_ = slice(g * HH * (NOPE + VH), (g + 1) * HH * (NOPE + VH))
        wqu_g = (Wqu[rq_, :] * gq[None, :] * scale)[perm_q]   # (1536, QL)
        bqu_g = (bqu[rq_] * scale)[perm_q]
        wkvu_g = (Wkvu[rkv_, :] * gkv[None, :])[perm_kv]      # (2048, KVL)
        bkvu_g = bkvu[rkv_][perm_kv]
        wo_g = Wo[:, g * HH * VH:(g + 1) * HH * VH]           # (D, 1024)
        tg = slice(g * TH, (g + 1) * TH)
        wkvu_t = wkvu_g.T                                     # (KVL, 2048)
        gmaps.append({
            "wqu": _sw(wqu_g.T, QC, QC, P),
            "bqu": np.ascontiguousarray(bqu_g.reshape(QC, P).T),
            "wkn": _sw(wkvu_t[:, :HH * P], HH, KC, P),
            "wv": _sw(wkvu_t[:, HH * P:], 4, KC, 256),
            "bkvuk": np.ascontiguousarray(
                bkvu_g[:HH * NOPE].reshape(HH, P).T),
            "wo": _sw(wo_g.T, DC, HH, P),
            "cosk": np.ascontiguousarray(cos[:, tg]).astype(bf),
            "sink": np.ascontiguousarray(sins[:, tg]).astype(bf),
        })

    # X with token axis reordered to [own half | peer half]
    xts = []
    for b in range(B):
        Xt = X[b].T
        xts.append([
            _sw(np.concatenate([Xt[:, :TH], Xt[:, TH:]], 1), 1, DC, T),
            _sw(np.concatenate([Xt[:, TH:], Xt[:, :TH]], 1), 1, DC, T),
        ])

    in_maps = []
    for c in range(8):
        b, g = c // 2, c % 2
        m = {
            "xt": xts[b][g], "wqd": wqd_g[g], "bqd": bqd_g[g],
            "wkvd": wkvd, "wkv5": wkv5, "bkvd": bkvd,
            "cos2": cos2, "sina": sina, "tri": tri,
        }
        m.update(gmaps[g])
        in_maps.append(m)
    return in_maps, bo_eff, start


def kernel(**inputs) -> np.ndarray:
    in_maps, bo_eff, start = _prep_inputs(**inputs)
    nc = _get_nc(start)
    try:
        res = run_bass_kernel_spmd(nc, in_maps, core_ids=list(range(8)))
    except Exception:
        res = run_bass_kernel_spmd(nc, in_maps, core_ids=list(range(8)))
    out = np.empty((B, S, D), np.float32)
    for b in range(B):
        acc = (res.results[2 * b]["outt"].astype(np.float32)
               + res.results[2 * b + 1]["outt"].astype(np.float32))
        out[b] = acc.T + bo_eff[None, :]
    return out


# revision 47
# speedup vs baseline: 1.0189x; 1.0189x over previous
"""Trainium2 Bass kernel for MultiHeadLatentAttention (MLA), 8-core SPMD.

Sharding: data-parallel over batch (4) x tensor-parallel over heads (2).
Core c handles batch c//2 and heads (c%2)*8 .. +8. Each core computes its
partial output projection; the host sums the two TP partials per batch and
adds the (v-bias-folded) output bias.

Device layout is feature-on-partition / token-on-free throughout, so every
projection is a plain matmul chain with no transposes. Attention uses
transposed scores (keys on partitions) so probs feed the AV matmul directly.

v3 notes (vs the 350us v2):
- BOTH down-projections are token-split across the TP pair: each core
  computes all output chunks for its own 512 tokens only. kv-down PE work
  halves; per-core X is 2MB instead of 4MB so the first matmul fires ~4us in.
- three pipelined AllGathers: kv latent + kRot early (hidden under the qd
  chains), then the raw q latent in two 6-chunk halves (hidden under
  kNope/v, feeding the qu chains just in time).
- q sum-of-squares pre-accumulated on the DVE (2 ones-matmuls instead of 24)
- q rot-half via gpsimd DMA block swaps instead of PE permutation matmuls
- v2 carry-overs: rsqrt via Sqrt+fast-reciprocal, head-pair score pipelining,
  v-bias folded into bo on the host, 128x128 universal triangle mask.
"""

import sys
from contextlib import ExitStack

import numpy as np
import ml_dtypes

for _p in ("/opt/trn_rl_repo", "/root/.axon_site/_ro/trn_rl_repo"):
    if _p not in sys.path:
        sys.path.append(_p)

import concourse.bass as bass  # noqa: E402
import concourse.mybir as mybir  # noqa: E402
from concourse import bacc  # noqa: E402
from concourse.bass_utils import run_bass_kernel_spmd  # noqa: E402
from concourse.tile import TileContext  # noqa: E402

# Problem shapes (hardcoded per contract)
B, S, D = 4, 1024, 2048
H = 16
QL, KVL = 1536, 512
NOPE, ROPE, VH = 128, 64, 128
QKH = NOPE + ROPE  # 192
EPS = 1e-6

P = 128
T = S          # tokens per core (one batch)
TH = T // 2    # own-token half per core
DC = D // P    # 16 X chunks
QC = QL // P   # 12 q-latent chunks
KC = KVL // P  # 4 kv-latent chunks
HH = H // 2    # 8 heads per core
NKV = KVL + ROPE  # 576
NEG = -1.0e4   # mask bias (exp underflows to exactly 0)

f32 = mybir.dt.float32
bf16 = mybir.dt.bfloat16
AF = mybir.ActivationFunctionType


def build_nc(start: int):
    nc = bacc.Bacc(None, target_bir_lowering=False, debug=False)

    # all weights arrive host-swizzled to partition-major tile layout
    # [P, tile, c, m] so every DMA descriptor is 1-4KB contiguous.
    # xt token axis is host-reordered to [own half | peer half] so the
    # token-split kv path is SPMD-uniform; qd is out-dim split (6 chunks).
    dp = nc.declare_dram_parameter
    xt = dp("xt", [P, DC * T], bf16, isOutput=False)      # X[b].T reordered
    wqd = dp("wqd", [P, 6 * DC * P], bf16, isOutput=False)
    wkvd = dp("wkvd", [P, 4 * DC * P], bf16, isOutput=False)
    wkv5 = dp("wkv5", [P, DC * ROPE], bf16, isOutput=False)
    wqu = dp("wqu", [P, QC * QC * P], bf16, isOutput=False)
    wkn = dp("wkn", [P, HH * KC * P], bf16, isOutput=False)
    wv = dp("wv", [P, 4 * KC * 256], bf16, isOutput=False)
    wo = dp("wo", [P, DC * HH * P], bf16, isOutput=False)
    bqd_i = dp("bqd", [P, 6], f32, isOutput=False)        # qd bias (local)
    bkvd_i = dp("bkvd", [P, 5], f32, isOutput=False)      # kv down bias
    bqu_i = dp("bqu", [P, QC], f32, isOutput=False)       # perm + scale
    bkvuk = dp("bkvuk", [P, HH], f32, isOutput=False)     # kNope part
    cos2 = dp("cos2", [P, T], bf16, isOutput=False)       # q rope, dup rows
    sina = dp("sina", [P, T], bf16, isOutput=False)       # sign-folded sin
    cosk = dp("cosk", [ROPE, TH], bf16, isOutput=False)   # k rope own tokens
    sink = dp("sink", [ROPE, TH], bf16, isOutput=False)   # sign-folded
    tri_i = dp("tri", [P, P], bf16, isOutput=False)       # diag-band mask
    outt = dp("outt", [D, T], bf16, isOutput=True)

    # collectives: kv latent+kRot early, q latent in two 3-chunk halves
    cc1_in = nc.dram_tensor("cc1_in", [5 * P, TH], bf16)
    cc1_out = nc.dram_tensor("cc1_out", [10 * P, TH], bf16)
    cc2a_in = nc.dram_tensor("cc2a_in", [3 * P, T], bf16)
    cc2a_out = nc.dram_tensor("cc2a_out", [6 * P, T], bf16)
    cc2b_in = nc.dram_tensor("cc2b_in", [3 * P, T], bf16)
    cc2b_out = nc.dram_tensor("cc2b_out", [6 * P, T], bf16)
    RG = [[0, 1], [2, 3], [4, 5], [6, 7]]

    xt_r = xt.rearrange("p (c t) -> p c t", c=DC)
    wqd_r = wqd.rearrange("p (n c m) -> p n c m", n=6, c=DC)
    wkvd_r = wkvd.rearrange("p (n c m) -> p n c m", n=4, c=DC)
    wkv5_r = wkv5.rearrange("p (c m) -> p c m", c=DC)
    wqu_r = wqu.rearrange("p (n c m) -> p n c m", n=QC, c=QC)
    wkn_r = wkn.rearrange("p (n c m) -> p n c m", n=HH, c=KC)
    wv_r = wv.rearrange("p (n c m) -> p n c m", n=4, c=KC)
    wo_r = wo.rearrange("p (n c m) -> p n c m", n=DC, c=HH)
    outt_r = outt.rearrange("(c p) t -> p c t", p=P)
    cc1_in_r = cc1_in.rearrange("(c p) t -> p c t", p=P)
    cc1_out_r = cc1_out.rearrange("(r c p) t -> p r c t", p=P, r=2)
    cc2a_in_r = cc2a_in.rearrange("(c p) t -> p c t", p=P)
    cc2a_out_r = cc2a_out.rearrange("(r c p) t -> p r c t", p=P, r=2)
    cc2b_in_r = cc2b_in.rearrange("(c p) t -> p c t", p=P)
    cc2b_out_r = cc2b_out.rearrange("(r c p) t -> p r c t", p=P, r=2)

    with TileContext(nc) as tc, ExitStack() as stk:
        const = stk.enter_context(tc.tile_pool(name="const", bufs=1))
        persist = stk.enter_context(tc.tile_pool(name="persist", bufs=1))

        # ---- constants in SBUF ----
        c_bqd = const.tile([P, 6], f32)
        c_bkvd = const.tile([P, 5], f32)
        c_bqu = const.tile([P, QC], f32)
        c_bkvuk = const.tile([P, HH], f32)
        c_tri = const.tile([P, P], bf16)
        c_cos = const.tile([P, T], bf16)
        c_sin = const.tile([P, T], bf16)
        c_cosk = const.tile([ROPE, TH], bf16)
        c_sink = const.tile([ROPE, TH], bf16)
        ones_bf = const.tile([P, P], bf16)
        nc.vector.memset(ones_bf[:], 1.0)
        eps_c = const.tile([P, 1], f32)
        nc.vector.memset(eps_c[:], EPS)

        # ---- persistent activations ----
        t_q = persist.tile([P, QC, T], bf16)      # q heads (nope 0-7, rope+)
        t_kn = persist.tile([P, HH, T], bf16)     # kNope[feat, head, tok]
        t_v = persist.tile([P, T // P, HH * P], bf16)  # v[tok, tchunk, hv]
        t_kr = persist.tile([P, T], bf16)         # kRot full, rows dup
        t_ao = persist.tile([P, HH, T], bf16)     # attn out [vh, head, tok]
        rq = persist.tile([P, T], f32)            # q rms scale (per token)

        # ====== phases 1+2: projections ======
        with tc.tile_pool(name="ph1", bufs=1) as ph1, \
             tc.tile_pool(name="wstream", bufs=2) as wst, \
             tc.tile_pool(name="wqu_p", bufs=2) as wqp, \
             tc.tile_pool(name="wkvu_p", bufs=2) as wkp, \
             tc.tile_pool(name="tmp", bufs=2) as tmp, \
             tc.tile_pool(name="psA", bufs=6, space="PSUM") as psA, \
             tc.tile_pool(name="psR", bufs=1, space="PSUM") as psR:

            # local (own-token / own-chunk) results stage into the gathered
            # tiles and are later overwritten by the identical gathered data
            t_x = ph1.tile([P, DC, T], bf16, name="t_x")
            t_kv = ph1.tile([P, KC, T], bf16, name="t_kv")     # gathered kv
            t_qd = ph1.tile([P, QC, T], bf16, name="t_qd")     # gathered q
            t_kp = ph1.tile([ROPE, TH], bf16, name="t_kp")     # rope chunk
            rkv = ph1.tile([P, TH], f32, name="rkv")
            acc = ph1.tile([P, 2, TH], bf16, name="acc")       # q ss preacc

            # X chunks + first kv weights lead. gpsimd carries ONLY the
            # collective traffic (+tiny consts) so cc1 fires asap.
            w_kvd = []
            for m in range(4):
                w_kvd.append(wst.tile([P, DC, P], bf16, tag="wkvd", bufs=4,
                                      name="w_kvd"))
            wt5 = wst.tile([P, DC, ROPE], bf16, tag="wkv5", bufs=1,
                           name="wt5")

            def wload(eng, wt, view, pieces=4):
                nchunk = wt.shape[1]
                step = (nchunk + pieces - 1) // pieces
                for i in range(0, nchunk, step):
                    j = min(i + step, nchunk)
                    eng.dma_start(wt[:, i:j, :], view[:, i:j, :])

            # preload the scalar activation table off the rms critical chain
            nc.scalar.activation(rkv[:, 0:1], eps_c[:], AF.Sqrt,
                                 bias=eps_c[:], scale=1.0)

            # issue order per queue == arrival order. Strict priority:
            # kv-down weights + own-half X first (kvd gates collective 1),
            # peer-half X next, qd weights stream after, cos/sin late.
            A, Bh = slice(0, TH), slice(TH, T)

            def xload(eng, c0, c1, h):
                eng.dma_start(t_x[:, c0:c1, h], xt_r[:, c0:c1, h])

            nc.gpsimd.dma_start(c_bkvd[:], bkvd_i[:])
            nc.gpsimd.dma_start(c_cosk[:], cosk[:])
            nc.gpsimd.dma_start(c_sink[:], sink[:])
            # kvd-critical set (own-half X + kv weight pieces), 3 queues,
            # ordered by the chunk-major consumption below
            def wpiece(eng, wt, view, j):
                eng.dma_start(wt[:, 4 * j:4 * j + 4, :],
                              view[:, 4 * j:4 * j + 4, :])

            for j in range(4):
                wpiece(nc.scalar, w_kvd[0], wkvd_r[:, 0], j)
                xload(nc.sync, 3 * j, 3 * j + 2, A)
                wpiece(nc.sync, w_kvd[1], wkvd_r[:, 1], j)
                wpiece(nc.gpsimd, w_kvd[2], wkvd_r[:, 2], j)
                xload(nc.scalar, 3 * j + 2, 3 * j + 3, A)
                wpiece(nc.scalar, w_kvd[3], wkvd_r[:, 3], j)
                xload(nc.gpsimd, 12 + j, 13 + j, A)
                nc.gpsimd.dma_start(wt5[:, 4 * j:4 * j + 4, :],
                                    wkv5_r[:, 4 * j:4 * j + 4, :])
            nc.gpsimd.dma_start(c_bqd[:], bqd_i[:])
            # peer-half X (qd tt=1) after everything kvd needs
            xload(nc.scalar, 0, 4, Bh)
            xload(nc.sync, 4, 8, Bh)
            xload(nc.scalar, 8, 12, Bh)
            xload(nc.sync, 12, 16, Bh)
            nc.gpsimd.dma_start(c_bqu[:], bqu_i[:])
            nc.gpsimd.dma_start(c_bkvuk[:], bkvuk[:])
            nc.gpsimd.dma_start(c_tri[:], tri_i[:])

            def down_chain(wt, m_rows, bias_t, bcol, out_ap, h):
                # out[m_rows, h] = wt.T @ X[:, h] + bias
                ps = psA.tile([P, TH], f32, tag="ev", name="ps_ev")
                psm = ps[:m_rows, :]
                for c in range(DC):
                    nc.tensor.matmul(
                        psm, wt[:, c, :m_rows], t_x[:, c, h],
                        start=(c == 0), stop=(c == DC - 1),
                    )
                nc.vector.tensor_scalar_add(
                    out=out_ap[:m_rows, :], in0=psm,
                    scalar1=bias_t[:m_rows, bcol:bcol + 1])

            # ---- kv down for own tokens (5 chunks incl. rope) ----
            # chunk-major across all 5 accumulation chains: the in-order PE
            # stream then only ever waits for X chunk c / weight piece c//4
            kvd_ps = []
            for m in range(5):
                kvd_ps.append(psA.tile([P, TH], f32, tag="ev",
                                       name="ps_kvd"))
            for c in range(DC):
                for m in range(5):
                    rows = P if m < 4 else ROPE
                    wt = w_kvd[m] if m < 4 else wt5
                    nc.tensor.matmul(
                        kvd_ps[m][:rows, :], wt[:, c, :rows], t_x[:, c, A],
                        start=(c == 0), stop=(c == DC - 1),
                    )
            for m in range(4):
                nc.vector.tensor_scalar_add(
                    out=t_kv[:, m, A], in0=kvd_ps[m][:],
                    scalar1=c_bkvd[:, m:m + 1])
            nc.vector.tensor_scalar_add(
                out=t_kp[:, :], in0=kvd_ps[4][:ROPE, :],
                scalar1=c_bkvd[:ROPE, 4:5])

            # kv rms on own tokens (ss via 4 ones-matmuls, then
            # Sqrt + fast-reciprocal + in-place normalize)
            ps_ms = psR.tile([P, 2, TH], f32, tag="ms", name="ps_ms")
            for c in range(KC):
                sq = tmp.tile([P, TH], bf16, tag="sq")
                nc.vector.tensor_mul(sq[:], t_kv[:, c, A], t_kv[:, c, A])
                nc.tensor.matmul(ps_ms[:, 0, :], ones_bf[:], sq[:],
                                 start=(c == 0), stop=(c == KC - 1))
            nc.scalar.activation(rkv[:], ps_ms[:, 0, :], AF.Sqrt,
                                 bias=eps_c[:], scale=1.0 / KVL)
            nc.vector.reciprocal_approx_fast(out=rkv[:], in_=rkv[:])
            for c in range(KC):
                nc.vector.tensor_mul(t_kv[:, c, A], t_kv[:, c, A],
                                     rkv[:])
            # RoPE on own kPos (unsigned 32-row swap + sign-folded sin)
            swp = tmp.tile([P, TH], bf16, tag="swp", name="swp",
                           bufs=1)[:ROPE, :]
            nc.sync.dma_start(swp[0:32, :], t_kp[32:64, :])
            nc.sync.dma_start(swp[32:64, :], t_kp[0:32, :])
            nc.vector.tensor_mul(t_kr[0:ROPE, A], t_kp[:, :], c_cosk[:])
            nc.vector.tensor_mul(swp[:], swp[:], c_sink[:])
            nc.vector.tensor_add(t_kr[0:ROPE, A], t_kr[0:ROPE, A],
                                 swp[:])
            nc.sync.dma_start(t_kr[ROPE:P, A], t_kr[0:ROPE, A])

            # ---- collective 1: normalized kv latent + kRot ----
            for m in range(4):
                nc.gpsimd.dma_start(cc1_in_r[:, m, :], t_kv[:, m, A])
            nc.gpsimd.dma_start(cc1_in_r[:, 4, :], t_kr[:, A])
            nc.gpsimd.collective_compute(
                "AllGather", mybir.AluOpType.bypass,
                replica_groups=RG,
                ins=[cc1_in[:]], outs=[cc1_out[:]],
            )
            for r in range(2):
                nc.gpsimd.dma_start(t_kv[:, 0:4, bass.ts(r, TH)],
                                    cc1_out_r[:, r, 0:4, :])
                nc.gpsimd.dma_start(t_kr[:, bass.ts(r, TH)],
                                    cc1_out_r[:, r, 4, :])

            # ---- q down: local 6 out-chunks over the full (reordered)
            # token axis; exchange in two 3-chunk collectives ----
            def q_exchange(cin_r, cin, cout, cout_r, m0):
                for mm in range(3):
                    nc.gpsimd.dma_start(cin_r[:, mm, :],
                                        t_qd[:, m0 + mm, :])
                nc.gpsimd.collective_compute(
                    "AllGather", mybir.AluOpType.bypass,
                    replica_groups=RG,
                    ins=[cin[:]], outs=[cout[:]],
                )
                # rank0 rows = global chunks m0..m0+2 (natural tokens);
                # rank1 rows = chunks m0+6.. with token halves swapped
                # (their producer's xt was reordered own-first too)
                nc.gpsimd.dma_start(t_qd[:, m0:m0 + 3, :],
                                    cout_r[:, 0, :, :])
                nc.gpsimd.dma_start(t_qd[:, m0 + 6:m0 + 9, A],
                                    cout_r[:, 1, :, TH:])
                nc.gpsimd.dma_start(t_qd[:, m0 + 6:m0 + 9, Bh],
                                    cout_r[:, 1, :, 0:TH])

            for m in range(6):
                wt = wst.tile([P, DC, P], bf16, tag="wqd", bufs=2,
                              name="w_qd")
                eng = nc.scalar if m % 2 == 0 else nc.sync
                wload(eng, wt, wqd_r[:, m])
                for tt in range(2):
                    down_chain(wt, P, c_bqd, m, t_qd[:, m, bass.ts(tt, TH)],
                               bass.ts(tt, TH))
                if m == 2:
                    q_exchange(cc2a_in_r, cc2a_in, cc2a_out, cc2a_out_r, 0)
            nc.scalar.dma_start(c_cos[:], cos2[:])
            nc.scalar.dma_start(c_sin[:], sina[:])
            q_exchange(cc2b_in_r, cc2b_in, cc2b_out, cc2b_out_r, 3)

            # ---- kNope up-projection (bias add on scalar engine) ----
            kn_w = []
            for m in range(HH):
                wt = wkp.tile([P, KC, P], bf16, tag="wkn", bufs=4,
                              name="kn_w")
                nc.sync.dma_start(wt[:], wkn_r[:, m])
                kn_w.append(wt)
            for m in range(HH):
                wt = kn_w[m]
                for tt in range(2):
                    ps = psA.tile([P, TH], f32, tag="ev", name="ps_kn")
                    for c in range(KC):
                        nc.tensor.matmul(
                            ps, wt[:, c, :],
                            t_kv[:, c, bass.ts(tt, TH)],
                            start=(c == 0), stop=(c == KC - 1),
                        )
                    nc.scalar.activation(
                        t_kn[:, m, bass.ts(tt, TH)], ps, AF.Identity,
                        bias=c_bkvuk[:, m:m + 1])

            # ---- v up-projection (token-on-partition) ----
            for gg in range(4):
                wt = wkp.tile([P, KC, 256], bf16, tag="wv")
                (nc.sync if gg % 2 == 0 else nc.scalar).dma_start(
                    wt[:], wv_r[:, gg])
                for tcb in range(8):
                    ps = psA.tile([P, 256], f32, tag="ev", name="ps_v")
                    for c in range(KC):
                        nc.tensor.matmul(
                            ps,
                            t_kv[:, c, bass.ts(tcb, P)],
                            wt[:, c, :],
                            start=(c == 0), stop=(c == KC - 1),
                        )
                    nc.scalar.activation(
                        t_v[:, tcb, bass.ds(gg * 256, 256)], ps, AF.Copy)

            # ---- q rms from the gathered raw latent (DVE pre-accum) ----
            with tc.tile_wait_until(0.085):
                for tt in range(2):
                    hs = bass.ts(tt, TH)
                    for c in range(QC):
                        if c == 0:
                            nc.vector.tensor_mul(
                                acc[:, tt, :], t_qd[:, 0, hs], t_qd[:, 0, hs])
                        else:
                            sq = tmp.tile([P, TH], bf16, tag="sq")
                            nc.vector.tensor_mul(
                                sq[:], t_qd[:, c, hs], t_qd[:, c, hs])
                            nc.vector.tensor_add(
                                acc[:, tt, :], acc[:, tt, :], sq[:])
                ps_mq = psR.tile([P, 2, TH], f32, tag="ms", name="ps_mq")
                for tt in range(2):
                    hs = bass.ts(tt, TH)
                    nc.tensor.matmul(ps_mq[:, tt, :], ones_bf[:],
                                     acc[:, tt, :], start=True, stop=True)
                    nc.scalar.activation(rq[:, hs], ps_mq[:, tt, :],
                                         AF.Sqrt, bias=eps_c[:],
                                         scale=1.0 / QL)
                    nc.vector.reciprocal_approx_fast(out=rq[:, hs],
                                                     in_=rq[:, hs])

            # ---- q up-projection ----
            # post-processing of chunk m's psums is issued after chunk m+1's
            # matmul chains, so the PE never waits on the DVE stage tiles
            def qu_post(m, ps, tt):
                tsl = bass.ts(tt, TH)
                if m < 8:
                    qsb = tmp.tile([P, TH], bf16, tag="qsb", bufs=2)
                    nc.vector.tensor_mul(qsb[:], ps, rq[:, tsl])
                    nc.scalar.activation(
                        t_q[:, m, tsl], qsb, AF.Identity,
                        bias=c_bqu[:, m:m + 1],
                    )
                else:
                    sq = tmp.tile([P, TH], bf16, tag="ropestage",
                                  bufs=2)
                    nc.vector.tensor_mul(sq[:], ps, rq[:, tsl])
                    nc.vector.tensor_scalar_add(
                        out=sq[:], in0=sq, scalar1=c_bqu[:, m:m + 1],
                    )
                    # rotate-half via gpsimd DMA 32-row block swaps
                    swb = tmp.tile([P, TH], bf16, tag="ropeswap",
                                   bufs=2)
                    nc.gpsimd.dma_start(swb[0:32, :], sq[32:64, :])
                    nc.gpsimd.dma_start(swb[32:64, :], sq[0:32, :])
                    nc.gpsimd.dma_start(swb[64:96, :], sq[96:128, :])
                    nc.gpsimd.dma_start(swb[96:128, :], sq[64:96, :])
                    qc = tmp.tile([P, TH], bf16, tag="ropecos", bufs=2)
                    nc.vector.tensor_mul(qc[:], sq[:], c_cos[:, tsl])
                    nc.vector.tensor_mul(swb[:], swb[:], c_sin[:, tsl])
                    nc.vector.tensor_add(t_q[:, m, tsl], qc[:], swb[:])

            pend = None
            for m in (8, 0, 1, 9, 2, 3, 10, 4, 5, 11, 6, 7):
                wt = wqp.tile([P, QC, P], bf16, tag="wqu")
                eng = nc.scalar if m % 2 == 0 else nc.sync
                wload(eng, wt, wqu_r[:, m])
                cur = []
                # contract the cc2a-delivered chunks first so the chains
                # tolerate a late cc2b readback
                CORD = (0, 1, 2, 6, 7, 8, 3, 4, 5, 9, 10, 11)
                for tt in range(2):
                    tsl = bass.ts(tt, TH)
                    ps = psA.tile([P, TH], f32, tag="ev", name="ps_qu")
                    for i, c in enumerate(CORD):
                        nc.tensor.matmul(
                            ps, wt[:, c, :], t_qd[:, c, tsl],
                            start=(i == 0), stop=(i == QC - 1),
                        )
                    cur.append(ps)
                if pend is not None:
                    pm, pps = pend
                    for tt in range(2):
                        qu_post(pm, pps[tt], tt)
                pend = (m, cur)
            pm, pps = pend
            for tt in range(2):
                qu_post(pm, pps[tt], tt)

        # ====== phase 3: attention (transposed scores, max-free) ======
        def vis_kcs(qt):
            return [kc for kc in range(8)
                    if qt * TH + TH - 1 >= kc * P - start]

        with tc.tile_pool(name="att", bufs=2) as att, \
             tc.tile_pool(name="psS", bufs=2, space="PSUM") as psS, \
             tc.tile_pool(name="psD", bufs=1, space="PSUM") as psD, \
             tc.tile_pool(name="psU", bufs=2, space="PSUM") as psU:

            def scores_qt(hp, expts2, qt, kcs=None):
                # expts2 [P, head2, kc, q] for heads (2hp, 2hp+1)
                rc = 8 + hp
                for kc in (vis_kcs(qt) if kcs is None else kcs):
                    lo = max(qt * TH, kc * P - start)
                    w = qt * TH + TH - lo
                    rel = lo - qt * TH
                    sc2 = psS.tile([P, 2, TH], f32, tag="sc", name="sc2")
                    for h2 in range(2):
                        h = 2 * hp + h2
                        nc.tensor.matmul(
                            sc2[:, h2, rel:],
                            t_kn[:, h, bass.ts(kc, P)],
                            t_q[:, h, bass.ds(lo, w)],
                            start=True, stop=False,
                        )
                    for h2 in range(2):
                        r0 = h2 * ROPE
                        nc.tensor.matmul(
                            sc2[:, h2, rel:],
                            t_kr[r0:r0 + ROPE, bass.ts(kc, P)],
                            t_q[r0:r0 + ROPE, rc, bass.ds(lo, w)],
                            start=False, stop=True,
                        )
                    # partially-masked diagonal band
                    b_lo = max(lo, kc * P - start)
                    b_hi = min(qt * TH + TH, kc * P - start + P)
                    bw = b_hi - b_lo
                    if bw > 0:
                        j0 = b_lo - (kc * P - start)
                        br = b_lo - qt * TH
                        for h2 in range(2):
                            nc.vector.tensor_add(
                                sc2[:, h2, br:br + bw],
                                sc2[:, h2, br:br + bw],
                                c_tri[:, j0:j0 + bw])
                    nc.scalar.activation(
                        expts2[:, :, kc, bass.ds(lo, w)],
                        sc2[:, :, rel:], AF.Exp)

            def den_head(hp, expts2, h2):
                den2 = psD.tile([P, 2, TH], f32, name="den2")
                for qt in range(2):
                    kcs = vis_kcs(qt)
                    for i, kc in enumerate(kcs):
                        lo = max(qt * TH, kc * P - start)
                        rel = lo - qt * TH
                        nc.tensor.matmul(
                            den2[:, qt, rel:], ones_bf[:],
                            expts2[:, h2, kc, bass.ds(lo, TH - rel)],
                            start=(i == 0), stop=(i == len(kcs) - 1),
                        )
                rcp = att.tile([P, 2, TH], f32, tag="rcp", name="rcp")
                nc.vector.reciprocal_approx_fast(
                    out=rcp[:, :, :], in_=den2[:, :, :])
                return rcp

            def outU_head(hp, expts2, h2, rcp):
                h = 2 * hp + h2
                for qt in range(2):
                    kcs = vis_kcs(qt)
                    outU = psU.tile([P, TH], f32, tag="outU", name="outU")
                    for i, kc in enumerate(kcs):
                        lo = max(qt * TH, kc * P - start)
                        rel = lo - qt * TH
                        nc.tensor.matmul(
                            outU[:, rel:], t_v[:, kc, bass.ts(h, P)],
                            expts2[:, h2, kc, bass.ds(lo, TH - rel)],
                            start=(i == 0), stop=(i == len(kcs) - 1),
                        )
                    nc.vector.tensor_mul(
                        t_ao[:, h, bass.ts(qt, TH)], outU[:],
                        rcp[:, qt, :])

            # interleave hp-1's den/outU chains between hp's score bursts
            # so the PE has filler while the exp stream catches up
            prev = None
            for hp in range(4):
                cur = att.tile([P, 2, 8, T], bf16, tag="expt", name="expt2")
                scores_qt(hp, cur, 0)
                if prev is not None:
                    rcp0 = den_head(hp - 1, prev, 0)
                scores_qt(hp, cur, 1, kcs=[0, 1, 2, 3])
                if prev is not None:
                    outU_head(hp - 1, prev, 0, rcp0)
                scores_qt(hp, cur, 1, kcs=[4, 5, 6, 7])
                if prev is not None:
                    rcp1 = den_head(hp - 1, prev, 1)
                    outU_head(hp - 1, prev, 1, rcp1)
                prev = cur
            for h2 in range(2):
                rcpt = den_head(3, prev, h2)
                outU_head(3, prev, h2, rcpt)

            # ====== phase 4: output projection ======
            for m in range(DC):
                wt = att.tile([P, HH, P], bf16, tag="wo", name="wo_t",
                              bufs=4)
                eng = nc.gpsimd if m % 2 == 0 else nc.sync
                eng.dma_start(wt[:], wo_r[:, m])
                for tt in range(2):
                    ps = psU.tile([P, TH], f32, tag="outU", name="ps_o")
                    for c in range(HH):
                        nc.tensor.matmul(
                            ps, wt[:, c, :], t_ao[:, c, bass.ts(tt, TH)],
                            start=(c == 0), stop=(c == HH - 1),
                        )
                    ot = att.tile([P, TH], bf16, tag="ot", name="ot",
                                  bufs=3)
                    nc.vector.tensor_copy(ot[:], ps)
                    oeng = nc.scalar if (2 * m + tt) % 2 == 0 else nc.sync
                    oeng.dma_start(outt_r[:, m, bass.ts(tt, TH)], ot[:])

    nc.compile()
    return nc


_CACHE = {}


def _get_nc(start: int):
    if start not in _CACHE:
        _CACHE[start] = build_nc(start)
    return _CACHE[start]


def _prep_inputs(X, base_freq, Wqd, bqd, gq, Wqu, bqu, Wkv, bkv, gkv,
                 Wkvu, bkvu, Wo, bo, start):
    f = np.float32
    X = np.asarray(X, f)
    base_freq = np.asarray(base_freq, f)
    Wqd = np.asarray(Wqd, f); bqd = np.asarray(bqd, f)
    gq = np.asarray(gq, f); Wqu = np.asarray(Wqu, f); bqu = np.asarray(bqu, f)
    Wkv = np.asarray(Wkv, f); bkv = np.asarray(bkv, f)
    gkv = np.asarray(gkv, f); Wkvu = np.asarray(Wkvu, f)
    bkvu = np.asarray(bkvu, f)
    Wo = np.asarray(Wo, f); bo = np.asarray(bo, f)
    start = int(np.asarray(start).item())
    assert start >= 0

    scale = QKH ** (-0.5)
    bf = ml_dtypes.bfloat16

    # v-bias exact fold: probs sum to 1, so the v bias contributes
    # Wo @ bv to every token's output.
    bv = bkvu.reshape(H, NOPE + VH)[:, NOPE:].reshape(H * VH)
    bo_eff = bo + Wo @ bv

    def _sw(wt, nt, c, m):
        # [c*P, nt*m] -> partition-major tiles [P, nt*c*m]
        a = np.asarray(wt, f).reshape(c, P, nt, m)
        a = np.ascontiguousarray(a.transpose(1, 2, 0, 3)).astype(bf)
        return a.reshape(P, nt * c * m)

    # qd is out-dim split: each core gets its group's 6 chunks
    wqd_g = [_sw(Wqd.T[:, g * 768:(g + 1) * 768], 6, DC, P) for g in range(2)]
    bqd_g = [np.ascontiguousarray(bqd[g * 768:(g + 1) * 768].reshape(6, P).T)
             for g in range(2)]
    wkv_t = Wkv.T.astype(f)                                   # (D, NKV)
    wkvd = _sw(wkv_t[:, :512], 4, DC, P)
    wkv5 = _sw(wkv_t[:, 512:576], 1, DC, ROPE)
    bkvd_p = np.zeros((5 * P,), f); bkvd_p[:NKV] = bkv
    bkvd = np.ascontiguousarray(bkvd_p.reshape(5, P).T)

    ang = base_freq[:S]                                       # (S, ROPE)
    cos = np.ascontiguousarray(np.cos(ang).T.astype(f))       # (ROPE, S)
    sin = np.ascontiguousarray(np.sin(ang).T.astype(f))
    cos2 = np.ascontiguousarray(
        np.concatenate([cos, cos], 0)).astype(bf)             # (128, S)
    sgn = np.ones((ROPE, 1), f); sgn[:ROPE // 2] = -1.0
    sins = sin * sgn                                          # sign-folded
    sina = np.ascontiguousarray(np.concatenate([sins, sins], 0)).astype(bf)

    # universal diagonal-band mask: for the block at k = kc*P + p,
    # q = (kc*P - start) + j, visibility is p <= j.
    pp = np.arange(P)
    tri = np.where(pp[:, None] <= pp[None, :], 0.0, NEG).astype(bf)
    tri = np.ascontiguousarray(tri)

    # per head-group tensors
    perm_q = np.concatenate(
        [np.arange(h * QKH, h * QKH + NOPE) for h in range(HH)]
        + [np.arange(h * QKH + NOPE, (h + 1) * QKH) for h in range(HH)]
    )
    perm_kv = np.concatenate(
        [np.arange(h * (NOPE + VH), h * (NOPE + VH) + NOPE) for h in range(HH)]
        + [np.arange(h * (NOPE + VH) + NOPE, (h + 1) * (NOPE + VH))
           for h in range(HH)]
    )
    gmaps = []
    for g in range(2):
        rq_ = slice(g * HH * QKH, (g + 1) * HH * QKH)
        rkv_ = slice(g * HH * (NOPE + VH), (g + 1) * HH * (NOPE + VH))
        wqu_g = (Wqu[rq_, :] * gq[None, :] * scale)[perm_q]   # (1536, QL)
        bqu_g = (bqu[rq_] * scale)[perm_q]
        wkvu_g = (Wkvu[rkv_, :] * gkv[None, :])[perm_kv]      # (2048, KVL)
        bkvu_g = bkvu[rkv_][perm_kv]
        wo_g = Wo[:, g * HH * VH:(g + 1) * HH * VH]           # (D, 1024)
        tg = slice(g * TH, (g + 1) * TH)
        wkvu_t = wkvu_g.T                                     # (KVL, 2048)
        gmaps.append({
            "wqu": _sw(wqu_g.T, QC, QC, P),
            "bqu": np.ascontiguousarray(bqu_g.reshape(QC, P).T),
            "wkn": _sw(wkvu_t[:, :HH * P], HH, KC, P),
            "wv": _sw(wkvu_t[:, HH * P:], 4, KC, 256),
            "bkvuk": np.ascontiguousarray(
                bkvu_g[:HH * NOPE].reshape(HH, P).T),
            "wo": _sw(wo_g.T, DC, HH, P),
            "cosk": np.ascontiguousarray(cos[:, tg]).astype(bf),
            "sink": np.ascontiguousarray(sins[:, tg]).astype(bf),
        })

    # X with token axis reordered to [own half | peer half]
    xts = []
    for b in range(B):
        Xt = X[b].T
        xts.append([
            _sw(np.concatenate([Xt[:, :TH], Xt[:, TH:]], 1), 1, DC, T),
            _sw(np.concatenate([Xt[:, TH:], Xt[:, :TH]], 1), 1, DC, T),
        ])

    in_maps = []
    for c in range(8):
        b, g = c // 2, c % 2
        m = {
            "xt": xts[b][g], "wqd": wqd_g[g], "bqd": bqd_g[g],
            "wkvd": wkvd, "wkv5": wkv5, "bkvd": bkvd,
            "cos2": cos2, "sina": sina, "tri": tri,
        }
        m.update(gmaps[g])
        in_maps.append(m)
    return in_maps, bo_eff, start


def kernel(**inputs) -> np.ndarray:
    in_maps, bo_eff, start = _prep_inputs(**inputs)
    nc = _get_nc(start)
    try:
        res = run_bass_kernel_spmd(nc, in_maps, core_ids=list(range(8)))
    except Exception:
        res = run_bass_kernel_spmd(nc, in_maps, core_ids=list(range(8)))
    out = np.empty((B, S, D), np.float32)
    for b in range(B):
        acc = (res.results[2 * b]["outt"].astype(np.float32)
               + res.results[2 * b + 1]["outt"].astype(np.float32))
        out[b] = acc.T + bo_eff[None, :]
    return out


# revision 52
# speedup vs baseline: 1.0367x; 1.0175x over previous
"""Trainium2 Bass kernel for MultiHeadLatentAttention (MLA), 8-core SPMD.

Sharding: data-parallel over batch (4) x tensor-parallel over heads (2).
Core c handles batch c//2 and heads (c%2)*8 .. +8. Each core computes its
partial output projection; the host sums the two TP partials per batch and
adds the (v-bias-folded) output bias.

Device layout is feature-on-partition / token-on-free throughout, so every
projection is a plain matmul chain with no transposes. Attention uses
transposed scores (keys on partitions) so probs feed the AV matmul directly.

v3 notes (vs the 350us v2):
- BOTH down-projections are token-split across the TP pair: each core
  computes all output chunks for its own 512 tokens only. kv-down PE work
  halves; per-core X is 2MB instead of 4MB so the first matmul fires ~4us in.
- three pipelined AllGathers: kv latent + kRot early (hidden under the qd
  chains), then the raw q latent in two 6-chunk halves (hidden under
  kNope/v, feeding the qu chains just in time).
- q sum-of-squares pre-accumulated on the DVE (2 ones-matmuls instead of 24)
- q rot-half via gpsimd DMA block swaps instead of PE permutation matmuls
- v2 carry-overs: rsqrt via Sqrt+fast-reciprocal, head-pair score pipelining,
  v-bias folded into bo on the host, 128x128 universal triangle mask.
"""

import sys
from contextlib import ExitStack

import numpy as np
import ml_dtypes

for _p in ("/opt/trn_rl_repo", "/root/.axon_site/_ro/trn_rl_repo"):
    if _p not in sys.path:
        sys.path.append(_p)

import concourse.bass as bass  # noqa: E402
import concourse.mybir as mybir  # noqa: E402
from concourse import bacc  # noqa: E402
from concourse.bass_utils import run_bass_kernel_spmd  # noqa: E402
from concourse.tile import TileContext  # noqa: E402

# Problem shapes (hardcoded per contract)
B, S, D = 4, 1024, 2048
H = 16
QL, KVL = 1536, 512
NOPE, ROPE, VH = 128, 64, 128
QKH = NOPE + ROPE  # 192
EPS = 1e-6

P = 128
T = S          # tokens per core (one batch)
TH = T // 2    # own-token half per core
DC = D // P    # 16 X chunks
QC = QL // P   # 12 q-latent chunks
KC = KVL // P  # 4 kv-latent chunks
HH = H // 2    # 8 heads per core
NKV = KVL + ROPE  # 576
NEG = -1.0e4   # mask bias (exp underflows to exactly 0)

f32 = mybir.dt.float32
bf16 = mybir.dt.bfloat16
AF = mybir.ActivationFunctionType


def build_nc(start: int):
    nc = bacc.Bacc(None, target_bir_lowering=False, debug=False)

    # all weights arrive host-swizzled to partition-major tile layout
    # [P, tile, c, m] so every DMA descriptor is 1-4KB contiguous.
    # xt token axis is host-reordered to [own half | peer half] so the
    # token-split kv path is SPMD-uniform; qd is out-dim split (6 chunks).
    dp = nc.declare_dram_parameter
    xt = dp("xt", [P, DC * T], bf16, isOutput=False)      # X[b].T reordered
    wqd = dp("wqd", [P, 6 * DC * P], bf16, isOutput=False)
    wkvd = dp("wkvd", [P, 4 * DC * P], bf16, isOutput=False)
    wkv5 = dp("wkv5", [P, DC * ROPE], bf16, isOutput=False)
    wqu = dp("wqu", [P, QC * QC * P], bf16, isOutput=False)
    wkn = dp("wkn", [P, HH * KC * P], bf16, isOutput=False)
    wv = dp("wv", [P, 4 * KC * 256], bf16, isOutput=False)
    wo = dp("wo", [P, DC * HH * P], bf16, isOutput=False)
    bqd_i = dp("bqd", [P, 6], f32, isOutput=False)        # qd bias (local)
    bkvd_i = dp("bkvd", [P, 5], f32, isOutput=False)      # kv down bias
    bqu_i = dp("bqu", [P, QC], f32, isOutput=False)       # perm + scale
    bkvuk = dp("bkvuk", [P, HH], f32, isOutput=False)     # kNope part
    cos2 = dp("cos2", [P, T], bf16, isOutput=False)       # q rope, dup rows
    sina = dp("sina", [P, T], bf16, isOutput=False)       # sign-folded sin
    cosk = dp("cosk", [ROPE, TH], bf16, isOutput=False)   # k rope own tokens
    sink = dp("sink", [ROPE, TH], bf16, isOutput=False)   # sign-folded
    tri_i = dp("tri", [P, P], bf16, isOutput=False)       # diag-band mask
    outt = dp("outt", [D, T], bf16, isOutput=True)

    # collectives: kv latent+kRot early, q latent in two 3-chunk halves
    cc1_in = nc.dram_tensor("cc1_in", [5 * P, TH], bf16)
    cc1_out = nc.dram_tensor("cc1_out", [10 * P, TH], bf16)
    cc2a_in = nc.dram_tensor("cc2a_in", [3 * P, T], bf16)
    cc2a_out = nc.dram_tensor("cc2a_out", [6 * P, T], bf16)
    cc2b_in = nc.dram_tensor("cc2b_in", [3 * P, T], bf16)
    cc2b_out = nc.dram_tensor("cc2b_out", [6 * P, T], bf16)
    RG = [[0, 1], [2, 3], [4, 5], [6, 7]]

    xt_r = xt.rearrange("p (c t) -> p c t", c=DC)
    wqd_r = wqd.rearrange("p (n c m) -> p n c m", n=6, c=DC)
    wkvd_r = wkvd.rearrange("p (n c m) -> p n c m", n=4, c=DC)
    wkv5_r = wkv5.rearrange("p (c m) -> p c m", c=DC)
    wqu_r = wqu.rearrange("p (n c m) -> p n c m", n=QC, c=QC)
    wkn_r = wkn.rearrange("p (n c m) -> p n c m", n=HH, c=KC)
    wv_r = wv.rearrange("p (n c m) -> p n c m", n=4, c=KC)
    wo_r = wo.rearrange("p (n c m) -> p n c m", n=DC, c=HH)
    outt_r = outt.rearrange("(c p) t -> p c t", p=P)
    cc1_in_r = cc1_in.rearrange("(c p) t -> p c t", p=P)
    cc1_out_r = cc1_out.rearrange("(r c p) t -> p r c t", p=P, r=2)
    cc2a_in_r = cc2a_in.rearrange("(c p) t -> p c t", p=P)
    cc2a_out_r = cc2a_out.rearrange("(r c p) t -> p r c t", p=P, r=2)
    cc2b_in_r = cc2b_in.rearrange("(c p) t -> p c t", p=P)
    cc2b_out_r = cc2b_out.rearrange("(r c p) t -> p r c t", p=P, r=2)

    with TileContext(nc) as tc, ExitStack() as stk:
        const = stk.enter_context(tc.tile_pool(name="const", bufs=1))
        persist = stk.enter_context(tc.tile_pool(name="persist", bufs=1))

        # ---- constants in SBUF ----
        c_bqd = const.tile([P, 6], f32)
        c_bkvd = const.tile([P, 5], f32)
        c_bqu = const.tile([P, QC], f32)
        c_bkvuk = const.tile([P, HH], f32)
        c_tri = const.tile([P, P], bf16)
        c_cos = const.tile([P, T], bf16)
        c_sin = const.tile([P, T], bf16)
        c_cosk = const.tile([ROPE, TH], bf16)
        c_sink = const.tile([ROPE, TH], bf16)
        ones_bf = const.tile([P, P], bf16)
        nc.vector.memset(ones_bf[:], 1.0)
        eps_c = const.tile([P, 1], f32)
        nc.vector.memset(eps_c[:], EPS)

        # ---- persistent activations ----
        t_q = persist.tile([P, QC, T], bf16)      # q heads (nope 0-7, rope+)
        t_kn = persist.tile([P, HH, T], bf16)     # kNope[feat, head, tok]
        t_v = persist.tile([P, T // P, HH * P], bf16)  # v[tok, tchunk, hv]
        t_kr = persist.tile([P, T], bf16)         # kRot full, rows dup
        t_ao = persist.tile([P, HH, T], bf16)     # attn out [vh, head, tok]
        rq = persist.tile([P, T], f32)            # q rms scale (per token)

        # ====== phases 1+2: projections ======
        with tc.tile_pool(name="ph1", bufs=1) as ph1, \
             tc.tile_pool(name="wstream", bufs=2) as wst, \
             tc.tile_pool(name="wqu_p", bufs=2) as wqp, \
             tc.tile_pool(name="wkvu_p", bufs=2) as wkp, \
             tc.tile_pool(name="tmp", bufs=2) as tmp, \
             tc.tile_pool(name="psA", bufs=6, space="PSUM") as psA, \
             tc.tile_pool(name="psR", bufs=1, space="PSUM") as psR:

            # local (own-token / own-chunk) results stage into the gathered
            # tiles and are later overwritten by the identical gathered data
            t_x = ph1.tile([P, DC, T], bf16, name="t_x")
            t_kv = ph1.tile([P, KC, T], bf16, name="t_kv")     # gathered kv
            t_qd = ph1.tile([P, QC, T], bf16, name="t_qd")     # gathered q
            t_kp = ph1.tile([ROPE, TH], bf16, name="t_kp")     # rope chunk
            rkv = ph1.tile([P, TH], f32, name="rkv")
            acc = ph1.tile([P, 2, TH], bf16, name="acc")       # q ss preacc

            # X chunks + first kv weights lead. gpsimd carries ONLY the
            # collective traffic (+tiny consts) so cc1 fires asap.
            w_kvd = []
            for m in range(4):
                w_kvd.append(wst.tile([P, DC, P], bf16, tag="wkvd", bufs=4,
                                      name="w_kvd"))
            wt5 = wst.tile([P, DC, ROPE], bf16, tag="wkv5", bufs=1,
                           name="wt5")

            def wload(eng, wt, view, pieces=4):
                nchunk = wt.shape[1]
                step = (nchunk + pieces - 1) // pieces
                for i in range(0, nchunk, step):
                    j = min(i + step, nchunk)
                    eng.dma_start(wt[:, i:j, :], view[:, i:j, :])

            # preload the scalar activation table off the rms critical chain
            nc.scalar.activation(rkv[:, 0:1], eps_c[:], AF.Sqrt,
                                 bias=eps_c[:], scale=1.0)

            # issue order per queue == arrival order. Strict priority:
            # kv-down weights + own-half X first (kvd gates collective 1),
            # peer-half X next, qd weights stream after, cos/sin late.
            A, Bh = slice(0, TH), slice(TH, T)

            def xload(eng, c0, c1, h):
                eng.dma_start(t_x[:, c0:c1, h], xt_r[:, c0:c1, h])

            nc.gpsimd.dma_start(c_bkvd[:], bkvd_i[:])
            nc.gpsimd.dma_start(c_cosk[:], cosk[:])
            nc.gpsimd.dma_start(c_sink[:], sink[:])
            # kvd-critical set (own-half X + kv weight pieces), 3 queues,
            # ordered by the chunk-major consumption below
            def wpiece(eng, wt, view, j):
                eng.dma_start(wt[:, 4 * j:4 * j + 4, :],
                              view[:, 4 * j:4 * j + 4, :])

            for j in range(4):
                wpiece(nc.scalar, w_kvd[0], wkvd_r[:, 0], j)
                xload(nc.sync, 3 * j, 3 * j + 2, A)
                wpiece(nc.sync, w_kvd[1], wkvd_r[:, 1], j)
                wpiece(nc.gpsimd, w_kvd[2], wkvd_r[:, 2], j)
                xload(nc.scalar, 3 * j + 2, 3 * j + 3, A)
                wpiece(nc.scalar, w_kvd[3], wkvd_r[:, 3], j)
                xload(nc.gpsimd, 12 + j, 13 + j, A)
                nc.gpsimd.dma_start(wt5[:, 4 * j:4 * j + 4, :],
                                    wkv5_r[:, 4 * j:4 * j + 4, :])
            nc.gpsimd.dma_start(c_bqd[:], bqd_i[:])
            # peer-half X (qd tt=1) after everything kvd needs
            xload(nc.scalar, 0, 4, Bh)
            xload(nc.sync, 4, 8, Bh)
            xload(nc.scalar, 8, 12, Bh)
            xload(nc.sync, 12, 16, Bh)
            nc.gpsimd.dma_start(c_bqu[:], bqu_i[:])
            nc.gpsimd.dma_start(c_bkvuk[:], bkvuk[:])
            nc.gpsimd.dma_start(c_tri[:], tri_i[:])

            def down_chain(wt, m_rows, bias_t, bcol, out_ap, h):
                # out[m_rows, h] = wt.T @ X[:, h] + bias
                ps = psA.tile([P, TH], f32, tag="ev", name="ps_ev")
                psm = ps[:m_rows, :]
                for c in range(DC):
                    nc.tensor.matmul(
                        psm, wt[:, c, :m_rows], t_x[:, c, h],
                        start=(c == 0), stop=(c == DC - 1),
                    )
                nc.vector.tensor_scalar_add(
                    out=out_ap[:m_rows, :], in0=psm,
                    scalar1=bias_t[:m_rows, bcol:bcol + 1])

            # ---- kv down for own tokens (5 chunks incl. rope) ----
            # chunk-major across all 5 accumulation chains: the in-order PE
            # stream then only ever waits for X chunk c / weight piece c//4
            kvd_ps = []
            for m in range(5):
                kvd_ps.append(psA.tile([P, TH], f32, tag="ev",
                                       name="ps_kvd"))
            for c in range(DC):
                for m in range(4):
                    nc.tensor.matmul(
                        kvd_ps[m][:], w_kvd[m][:, c, :], t_x[:, c, A],
                        start=(c == 0), stop=(c == DC - 1),
                    )
            # latent chains drain + rms while the rope chain still runs
            ps_ms = psR.tile([P, 2, TH], f32, tag="ms", name="ps_ms")
            for m in range(4):
                nc.vector.tensor_scalar_add(
                    out=t_kv[:, m, A], in0=kvd_ps[m][:],
                    scalar1=c_bkvd[:, m:m + 1])
                sq = tmp.tile([P, TH], bf16, tag="sq")
                nc.vector.tensor_mul(sq[:], t_kv[:, m, A], t_kv[:, m, A])
                nc.tensor.matmul(ps_ms[:, 0, :], ones_bf[:], sq[:],
                                 start=(m == 0), stop=(m == 3))
            for c in range(DC):
                nc.tensor.matmul(
                    kvd_ps[4][:ROPE, :], wt5[:, c, :], t_x[:, c, A],
                    start=(c == 0), stop=(c == DC - 1),
                )
            nc.vector.tensor_scalar_add(
                out=t_kp[:, :], in0=kvd_ps[4][:ROPE, :],
                scalar1=c_bkvd[:ROPE, 4:5])
            nc.scalar.activation(rkv[:], ps_ms[:, 0, :], AF.Sqrt,
                                 bias=eps_c[:], scale=1.0 / KVL)
            nc.vector.reciprocal_approx_fast(out=rkv[:], in_=rkv[:])
            for c in range(KC):
                nc.vector.tensor_mul(t_kv[:, c, A], t_kv[:, c, A],
                                     rkv[:])
            # RoPE on own kPos (unsigned 32-row swap + sign-folded sin)
            swp = tmp.tile([P, TH], bf16, tag="swp", name="swp",
                           bufs=1)[:ROPE, :]
            nc.sync.dma_start(swp[0:32, :], t_kp[32:64, :])
            nc.sync.dma_start(swp[32:64, :], t_kp[0:32, :])
            nc.vector.tensor_mul(t_kr[0:ROPE, A], t_kp[:, :], c_cosk[:])
            nc.vector.tensor_mul(swp[:], swp[:], c_sink[:])
            nc.vector.tensor_add(t_kr[0:ROPE, A], t_kr[0:ROPE, A],
                                 swp[:])
            nc.sync.dma_start(t_kr[ROPE:P, A], t_kr[0:ROPE, A])

            # ---- collective 1: normalized kv latent + kRot ----
            for m in range(4):
                eng = nc.sync if m % 2 else nc.gpsimd
                eng.dma_start(cc1_in_r[:, m, :], t_kv[:, m, A])
            nc.gpsimd.dma_start(cc1_in_r[:, 4, :], t_kr[:, A])
            nc.gpsimd.collective_compute(
                "AllGather", mybir.AluOpType.bypass,
                replica_groups=RG,
                ins=[cc1_in[:]], outs=[cc1_out[:]],
            )
            for r in range(2):
                nc.gpsimd.dma_start(t_kv[:, 0:4, bass.ts(r, TH)],
                                    cc1_out_r[:, r, 0:4, :])
                nc.gpsimd.dma_start(t_kr[:, bass.ts(r, TH)],
                                    cc1_out_r[:, r, 4, :])

            # ---- q down: local 6 out-chunks over the full (reordered)
            # token axis; exchange in two 3-chunk collectives ----
            def q_exchange(cin_r, cin, cout, cout_r, m0):
                for mm in range(3):
                    nc.gpsimd.dma_start(cin_r[:, mm, :],
                                        t_qd[:, m0 + mm, :])
                nc.gpsimd.collective_compute(
                    "AllGather", mybir.AluOpType.bypass,
                    replica_groups=RG,
                    ins=[cin[:]], outs=[cout[:]],
                )
                # rank0 rows = global chunks m0..m0+2 (natural tokens);
                # rank1 rows = chunks m0+6.. with token halves swapped
                # (their producer's xt was reordered own-first too)
                nc.gpsimd.dma_start(t_qd[:, m0:m0 + 3, :],
                                    cout_r[:, 0, :, :])
                nc.gpsimd.dma_start(t_qd[:, m0 + 6:m0 + 9, A],
                                    cout_r[:, 1, :, TH:])
                nc.gpsimd.dma_start(t_qd[:, m0 + 6:m0 + 9, Bh],
                                    cout_r[:, 1, :, 0:TH])

            for m in range(6):
                wt = wst.tile([P, DC, P], bf16, tag="wqd", bufs=2,
                              name="w_qd")
                eng = nc.scalar if m % 2 == 0 else nc.sync
                wload(eng, wt, wqd_r[:, m])
                for tt in range(2):
                    down_chain(wt, P, c_bqd, m, t_qd[:, m, bass.ts(tt, TH)],
                               bass.ts(tt, TH))
                if m == 2:
                    q_exchange(cc2a_in_r, cc2a_in, cc2a_out, cc2a_out_r, 0)
            nc.scalar.dma_start(c_cos[:], cos2[:])
            nc.scalar.dma_start(c_sin[:], sina[:])
            q_exchange(cc2b_in_r, cc2b_in, cc2b_out, cc2b_out_r, 3)

            # ---- kNope up-projection (bias add on scalar engine) ----
            kn_w = []
            for m in range(HH):
                wt = wkp.tile([P, KC, P], bf16, tag="wkn", bufs=8,
                              name="kn_w")
                nc.sync.dma_start(wt[:], wkn_r[:, m])
                kn_w.append(wt)
            for m in range(HH):
                wt = kn_w[m]
                for tt in range(2):
                    ps = psA.tile([P, TH], f32, tag="ev", name="ps_kn")
                    for c in range(KC):
                        nc.tensor.matmul(
                            ps, wt[:, c, :],
                            t_kv[:, c, bass.ts(tt, TH)],
                            start=(c == 0), stop=(c == KC - 1),
                        )
                    nc.scalar.activation(
                        t_kn[:, m, bass.ts(tt, TH)], ps, AF.Identity,
                        bias=c_bkvuk[:, m:m + 1])

            # ---- v up-projection (token-on-partition) ----
            for gg in range(4):
                wt = wkp.tile([P, KC, 256], bf16, tag="wv")
                (nc.sync if gg % 2 == 0 else nc.scalar).dma_start(
                    wt[:], wv_r[:, gg])
                for tcb in range(8):
                    ps = psA.tile([P, 256], f32, tag="ev", name="ps_v")
                    for c in range(KC):
                        nc.tensor.matmul(
                            ps,
                            t_kv[:, c, bass.ts(tcb, P)],
                            wt[:, c, :],
                            start=(c == 0), stop=(c == KC - 1),
                        )
                    nc.scalar.activation(
                        t_v[:, tcb, bass.ds(gg * 256, 256)], ps, AF.Copy)

            # ---- q rms from the gathered raw latent (DVE pre-accum) ----
            with tc.tile_wait_until(0.085):
                for tt in range(2):
                    hs = bass.ts(tt, TH)
                    for c in range(QC):
                        if c == 0:
                            nc.vector.tensor_mul(
                                acc[:, tt, :], t_qd[:, 0, hs], t_qd[:, 0, hs])
                        else:
                            sq = tmp.tile([P, TH], bf16, tag="sq")
                            nc.vector.tensor_mul(
                                sq[:], t_qd[:, c, hs], t_qd[:, c, hs])
                            nc.vector.tensor_add(
                                acc[:, tt, :], acc[:, tt, :], sq[:])
                ps_mq = psR.tile([P, 2, TH], f32, tag="ms", name="ps_mq")
                for tt in range(2):
                    hs = bass.ts(tt, TH)
                    nc.tensor.matmul(ps_mq[:, tt, :], ones_bf[:],
                                     acc[:, tt, :], start=True, stop=True)
                    nc.scalar.activation(rq[:, hs], ps_mq[:, tt, :],
                                         AF.Sqrt, bias=eps_c[:],
                                         scale=1.0 / QL)
                    nc.vector.reciprocal_approx_fast(out=rq[:, hs],
                                                     in_=rq[:, hs])

            # ---- q up-projection ----
            # post-processing of chunk m's psums is issued after chunk m+1's
            # matmul chains, so the PE never waits on the DVE stage tiles
            def qu_post(m, ps, tt):
                tsl = bass.ts(tt, TH)
                if m < 8:
                    qsb = tmp.tile([P, TH], bf16, tag="qsb", bufs=2)
                    nc.vector.tensor_mul(qsb[:], ps, rq[:, tsl])
                    nc.scalar.activation(
                        t_q[:, m, tsl], qsb, AF.Identity,
                        bias=c_bqu[:, m:m + 1],
                    )
                else:
                    sq = tmp.tile([P, TH], bf16, tag="ropestage",
                                  bufs=2)
                    nc.vector.tensor_mul(sq[:], ps, rq[:, tsl])
                    nc.vector.tensor_scalar_add(
                        out=sq[:], in0=sq, scalar1=c_bqu[:, m:m + 1],
                    )
                    # rotate-half via gpsimd DMA 32-row block swaps
                    swb = tmp.tile([P, TH], bf16, tag="ropeswap",
                                   bufs=2)
                    nc.gpsimd.dma_start(swb[0:32, :], sq[32:64, :])
                    nc.gpsimd.dma_start(swb[32:64, :], sq[0:32, :])
                    nc.gpsimd.dma_start(swb[64:96, :], sq[96:128, :])
                    nc.gpsimd.dma_start(swb[96:128, :], sq[64:96, :])
                    qc = tmp.tile([P, TH], bf16, tag="ropecos", bufs=1)
                    nc.vector.tensor_mul(qc[:], sq[:], c_cos[:, tsl])
                    nc.vector.tensor_mul(swb[:], swb[:], c_sin[:, tsl])
                    nc.vector.tensor_add(t_q[:, m, tsl], qc[:], swb[:])

            pend = None
            for m in (8, 0, 1, 9, 2, 3, 10, 4, 5, 11, 6, 7):
                wt = wqp.tile([P, QC, P], bf16, tag="wqu")
                eng = nc.scalar if m % 2 == 0 else nc.sync
                wload(eng, wt, wqu_r[:, m])
                cur = []
                # contract the cc2a-delivered chunks first so the chains
                # tolerate a late cc2b readback
                CORD = (0, 1, 2, 6, 7, 8, 3, 4, 5, 9, 10, 11)
                for tt in range(2):
                    tsl = bass.ts(tt, TH)
                    ps = psA.tile([P, TH], f32, tag="ev", name="ps_qu")
                    for i, c in enumerate(CORD):
                        nc.tensor.matmul(
                            ps, wt[:, c, :], t_qd[:, c, tsl],
                            start=(i == 0), stop=(i == QC - 1),
                        )
                    cur.append(ps)
                if pend is not None:
                    pm, pps = pend
                    for tt in range(2):
                        qu_post(pm, pps[tt], tt)
                pend = (m, cur)
            pm, pps = pend
            for tt in range(2):
                qu_post(pm, pps[tt], tt)

        # ====== phase 3: attention (transposed scores, max-free) ======
        def vis_kcs(qt):
            return [kc for kc in range(8)
                    if qt * TH + TH - 1 >= kc * P - start]

        with tc.tile_pool(name="att", bufs=2) as att, \
             tc.tile_pool(name="psS", bufs=2, space="PSUM") as psS, \
             tc.tile_pool(name="psD", bufs=1, space="PSUM") as psD, \
             tc.tile_pool(name="psU", bufs=2, space="PSUM") as psU:

            def scores_qt(hp, expts2, qt, kcs=None):
                # expts2 [P, head2, kc, q] for heads (2hp, 2hp+1)
                rc = 8 + hp
                for kc in (vis_kcs(qt) if kcs is None else kcs):
                    lo = max(qt * TH, kc * P - start)
                    w = qt * TH + TH - lo
                    rel = lo - qt * TH
                    sc2 = psS.tile([P, 2, TH], f32, tag="sc", name="sc2")
                    for h2 in range(2):
                        h = 2 * hp + h2
                        nc.tensor.matmul(
                            sc2[:, h2, rel:],
                            t_kn[:, h, bass.ts(kc, P)],
                            t_q[:, h, bass.ds(lo, w)],
                            start=True, stop=False,
                        )
                    for h2 in range(2):
                        r0 = h2 * ROPE
                        nc.tensor.matmul(
                            sc2[:, h2, rel:],
                            t_kr[r0:r0 + ROPE, bass.ts(kc, P)],
                            t_q[r0:r0 + ROPE, rc, bass.ds(lo, w)],
                            start=False, stop=True,
                        )
                    # partially-masked diagonal band
                    b_lo = max(lo, kc * P - start)
                    b_hi = min(qt * TH + TH, kc * P - start + P)
                    bw = b_hi - b_lo
                    if bw > 0:
                        j0 = b_lo - (kc * P - start)
                        br = b_lo - qt * TH
                        for h2 in range(2):
                            nc.vector.tensor_add(
                                sc2[:, h2, br:br + bw],
                                sc2[:, h2, br:br + bw],
                                c_tri[:, j0:j0 + bw])
                    nc.scalar.activation(
                        expts2[:, :, kc, bass.ds(lo, w)],
                        sc2[:, :, rel:], AF.Exp)

            def den_head(hp, expts2, h2):
                den2 = psD.tile([P, 2, TH], f32, name="den2")
                for qt in range(2):
                    kcs = vis_kcs(qt)
                    for i, kc in enumerate(kcs):
                        lo = max(qt * TH, kc * P - start)
                        rel = lo - qt * TH
                        nc.tensor.matmul(
                            den2[:, qt, rel:], ones_bf[:],
                            expts2[:, h2, kc, bass.ds(lo, TH - rel)],
                            start=(i == 0), stop=(i == len(kcs) - 1),
                        )
                rcp = att.tile([P, 2, TH], f32, tag="rcp", name="rcp")
                nc.vector.reciprocal_approx_fast(
                    out=rcp[:, :, :], in_=den2[:, :, :])
                return rcp

            def outU_head(hp, expts2, h2, rcp):
                h = 2 * hp + h2
                for qt in range(2):
                    kcs = vis_kcs(qt)
                    outU = psU.tile([P, TH], f32, tag="outU", name="outU")
                    for i, kc in enumerate(kcs):
                        lo = max(qt * TH, kc * P - start)
                        rel = lo - qt * TH
                        nc.tensor.matmul(
                            outU[:, rel:], t_v[:, kc, bass.ts(h, P)],
                            expts2[:, h2, kc, bass.ds(lo, TH - rel)],
                            start=(i == 0), stop=(i == len(kcs) - 1),
                        )
                    nc.vector.tensor_mul(
                        t_ao[:, h, bass.ts(qt, TH)], outU[:],
                        rcp[:, qt, :])

            # interleave hp-1's den/outU chains between hp's score bursts
            # so the PE has filler while the exp stream catches up
            prev = None
            for hp in range(4):
                cur = att.tile([P, 2, 8, T], bf16, tag="expt", name="expt2")
                scores_qt(hp, cur, 0)
                if prev is not None:
                    rcp0 = den_head(hp - 1, prev, 0)
                scores_qt(hp, cur, 1, kcs=[0, 1, 2, 3])
                if prev is not None:
                    outU_head(hp - 1, prev, 0, rcp0)
                scores_qt(hp, cur, 1, kcs=[4, 5, 6, 7])
                if prev is not None:
                    rcp1 = den_head(hp - 1, prev, 1)
                    outU_head(hp - 1, prev, 1, rcp1)
                prev = cur
            for h2 in range(2):
                rcpt = den_head(3, prev, h2)
                outU_head(3, prev, h2, rcpt)

            # ====== phase 4: output projection ======
            for m in range(DC):
                wt = att.tile([P, HH, P], bf16, tag="wo", name="wo_t",
                              bufs=4)
                eng = nc.gpsimd if m % 2 == 0 else nc.sync
                eng.dma_start(wt[:], wo_r[:, m])
                for tt in range(2):
                    ps = psU.tile([P, TH], f32, tag="outU", name="ps_o")
                    for c in range(HH):
                        nc.tensor.matmul(
                            ps, wt[:, c, :], t_ao[:, c, bass.ts(tt, TH)],
                            start=(c == 0), stop=(c == HH - 1),
                        )
                    ot = att.tile([P, TH], bf16, tag="ot", name="ot",
                                  bufs=3)
                    nc.vector.tensor_copy(ot[:], ps)
                    oeng = nc.scalar if (2 * m + tt) % 2 == 0 else nc.sync
                    oeng.dma_start(outt_r[:, m, bass.ts(tt, TH)], ot[:])

    nc.compile()
    return nc


_CACHE = {}


def _get_nc(start: int):
    if start not in _CACHE:
        _CACHE[start] = build_nc(start)
    return _CACHE[start]


def _prep_inputs(X, base_freq, Wqd, bqd, gq, Wqu, bqu, Wkv, bkv, gkv,
                 Wkvu, bkvu, Wo, bo, start):
    f = np.float32
    X = np.asarray(X, f)
    base_freq = np.asarray(base_freq, f)
    Wqd = np.asarray(Wqd, f); bqd = np.asarray(bqd, f)
    gq = np.asarray(gq, f); Wqu = np.asarray(Wqu, f); bqu = np.asarray(bqu, f)
    Wkv = np.asarray(Wkv, f); bkv = np.asarray(bkv, f)
    gkv = np.asarray(gkv, f); Wkvu = np.asarray(Wkvu, f)
    bkvu = np.asarray(bkvu, f)
    Wo = np.asarray(Wo, f); bo = np.asarray(bo, f)
    start = int(np.asarray(start).item())
    assert start >= 0

    scale = QKH ** (-0.5)
    bf = ml_dtypes.bfloat16

    # v-bias exact fold: probs sum to 1, so the v bias contributes
    # Wo @ bv to every token's output.
    bv = bkvu.reshape(H, NOPE + VH)[:, NOPE:].reshape(H * VH)
    bo_eff = bo + Wo @ bv

    def _sw(wt, nt, c, m):
        # [c*P, nt*m] -> partition-major tiles [P, nt*c*m]
        a = np.asarray(wt, f).reshape(c, P, nt, m)
        a = np.ascontiguousarray(a.transpose(1, 2, 0, 3)).astype(bf)
        return a.reshape(P, nt * c * m)

    # qd is out-dim split: each core gets its group's 6 chunks
    wqd_g = [_sw(Wqd.T[:, g * 768:(g + 1) * 768], 6, DC, P) for g in range(2)]
    bqd_g = [np.ascontiguousarray(bqd[g * 768:(g + 1) * 768].reshape(6, P).T)
             for g in range(2)]
    wkv_t = Wkv.T.astype(f)                                   # (D, NKV)
    wkvd = _sw(wkv_t[:, :512], 4, DC, P)
    wkv5 = _sw(wkv_t[:, 512:576], 1, DC, ROPE)
    bkvd_p = np.zeros((5 * P,), f); bkvd_p[:NKV] = bkv
    bkvd = np.ascontiguousarray(bkvd_p.reshape(5, P).T)

    ang = base_freq[:S]                                       # (S, ROPE)
    cos = np.ascontiguousarray(np.cos(ang).T.astype(f))       # (ROPE, S)
    sin = np.ascontiguousarray(np.sin(ang).T.astype(f))
    cos2 = np.ascontiguousarray(
        np.concatenate([cos, cos], 0)).astype(bf)             # (128, S)
    sgn = np.ones((ROPE, 1), f); sgn[:ROPE // 2] = -1.0
    sins = sin * sgn                                          # sign-folded
    sina = np.ascontiguousarray(np.concatenate([sins, sins], 0)).astype(bf)

    # universal diagonal-band mask: for the block at k = kc*P + p,
    # q = (kc*P - start) + j, visibility is p <= j.
    pp = np.arange(P)
    tri = np.where(pp[:, None] <= pp[None, :], 0.0, NEG).astype(bf)
    tri = np.ascontiguousarray(tri)

    # per head-group tensors
    perm_q = np.concatenate(
        [np.arange(h * QKH, h * QKH + NOPE) for h in range(HH)]
        + [np.arange(h * QKH + NOPE, (h + 1) * QKH) for h in range(HH)]
    )
    perm_kv = np.concatenate(
        [np.arange(h * (NOPE + VH), h * (NOPE + VH) + NOPE) for h in range(HH)]
        + [np.arange(h * (NOPE + VH) + NOPE, (h + 1) * (NOPE + VH))
           for h in range(HH)]
    )
    gmaps = []
    for g in range(2):
        rq_ = slice(g * HH * QKH, (g + 1) * HH * QKH)
        rkv_ = slice(g * HH * (NOPE + VH), (g + 1) * HH * (NOPE + VH))
        wqu_g = (Wqu[rq_, :] * gq[None, :] * scale)[perm_q]   # (1536, QL)
        bqu_g = (bqu[rq_] * scale)[perm_q]
        wkvu_g = (Wkvu[rkv_, :] * gkv[None, :])[perm_kv]      # (2048, KVL)
        bkvu_g = bkvu[rkv_][perm_kv]
        wo_g = Wo[:, g * HH * VH:(g + 1) * HH * VH]           # (D, 1024)
        tg = slice(g * TH, (g + 1) * TH)
        wkvu_t = wkvu_g.T                                     # (KVL, 2048)
        gmaps.append({
            "wqu": _sw(wqu_g.T, QC, QC, P),
            "bqu": np.ascontiguousarray(bqu_g.reshape(QC, P).T),
            "wkn": _sw(wkvu_t[:, :HH * P], HH, KC, P),
            "wv": _sw(wkvu_t[:, HH * P:], 4, KC, 256),
            "bkvuk": np.ascontiguousarray(
                bkvu_g[:HH * NOPE].reshape(HH, P).T),
            "wo": _sw(wo_g.T, DC, HH, P),
            "cosk": np.ascontiguousarray(cos[:, tg]).astype(bf),
            "sink": np.ascontiguousarray(sins[:, tg]).astype(bf),
        })

    # X with token axis reordered to [own half | peer half]
    xts = []
    for b in range(B):
        Xt = X[b].T
        xts.append([
            _sw(np.concatenate([Xt[:, :TH], Xt[:, TH:]], 1), 1, DC, T),
            _sw(np.concatenate([Xt[:, TH:], Xt[:, :TH]], 1), 1, DC, T),
        ])

    in_maps = []
    for c in range(8):
        b, g = c // 2, c % 2
        m = {
            "xt": xts[b][g], "wqd": wqd_g[g], "bqd": bqd_g[g],
            "wkvd": wkvd, "wkv5": wkv5, "bkvd": bkvd,
            "cos2": cos2, "sina": sina, "tri": tri,
        }
        m.update(gmaps[g])
        in_maps.append(m)
    return in_maps, bo_eff, start


def kernel(**inputs) -> np.ndarray:
    in_maps, bo_eff, start = _prep_inputs(**inputs)
    nc = _get_nc(start)
    try:
        res = run_bass_kernel_spmd(nc, in_maps, core_ids=list(range(8)))
    except Exception:
        res = run_bass_kernel_spmd(nc, in_maps, core_ids=list(range(8)))
    out = np.empty((B, S, D), np.float32)
    for b in range(B):
        acc = (res.results[2 * b]["outt"].astype(np.float32)
               + res.results[2 * b + 1]["outt"].astype(np.float32))
        out[b] = acc.T + bo_eff[None, :]
    return out
